# revision 22
# baseline (speedup 1.0000x reference)
"""Trainium2 Bass kernel for nn_LogicConv3d (differentiable-logic conv tree).

Problem (hardcoded): x [16,64,32,32] f32; idx_a/idx_b [64,900,64,3] i32;
w0..w6 [s,64,16] f32 (s = 64,32,16,8,4,2,1). Output [16,64,900,1] f32.

Math: per (kernel k, window p): gather 64 (a,b) leaf pairs from x, blend each
pair with soft-gate coefficients (softmax(w) @ GATE_M), then 6 more pairwise
tree levels.  mix(a,b) = c0 + c1*a + c2*b + c3*a*b.

v3 design (fp16 end-to-end, DVE 2x/4x perf modes):
 - F-sharding: core i handles batches (2i, 2i+1); pure SPMD across 8 cores.
 - Host builds a 576-row fp16 crop table XS[(c,ha,wa), 1920]: row = the
   30x30x2 (h,w,b-interleaved) crop of channel c at shift (ha,wa), compact in
   the first 1800 elements.  Leaf gathers are indirect DMAs with per-lane
   element offsets row*1920, fetching 1800 contiguous fp16 -> operands are
   step-1 fp16, which unlocks DVE packed modes.
 - scalar_tensor_tensor has NO DVE perf modes (1x only) so the mix avoids it:
     p = a*alpha + beta   (tensor_scalar 4x on DVE, or ACT activation)
     q = b * p            (tensor_tensor mult, 2x on DVE; some on Pool)
     u = a*gamma          (tensor_scalar / ACT)
     out = q + u          (tensor_tensor add, 2x on DVE)
   alpha = c3, beta = c2 - c3*Ta, gamma = c1 - c3*Tb where Ta/Tb are the
   children's additive-bias chain (bias folding; all multiplicative, safe).
   The per-node bias T = c2*Tb + c1*Ta - c3*Ta*Tb - c0 propagates on host in
   f64; the root bias is subtracted on host.
"""
import numpy as np

B, C, H, W = 16, 64, 32, 32
K = 64
RF = 3
DEPTH = 6
S = 64
PW = 30            # windows per axis
P = PW * PW        # 900
NCORES = 8
B2 = 2             # batches per core
F = P * B2         # free size (windows x batches) = 1800
NROW = C * RF * RF          # 576 crop-table rows
ROWE = 1920                 # crop-table row stride (elements)

# static engine assignment (tuned against the HW trace; see LP in notes):
#  - all p-ops on ACT
#  - q-TT: Pool for i%16 in [0,7), else DVE
#  - u+add: fused stt on DVE, except i%4==0 mixes use CCE-DMA add with a
#    separate u tile (u on ACT, or DVE-ts when i%16==0)
def _q_on_pool(i):
    return i % 9 < 2          # ~14 q-TT ops on Pool


def _use_cce(i, level):
    return i % 8 < 3 and level < 6   # 24 mixes: add via CCE-DMA


def _u_on_act(i):
    return True               # u for CCE mixes always on ACT


def _p_on_act(i):
    return i % 8 != 3         # 56 p-ops on ACT, 8 on DVE-ts


NCOL_GATHER = 1      # L0 ops batched per indirect gather DMA (1 = per-side)

GATE_M = np.array([
    [0, 0, 0, 0], [0, 0, 0, 1], [0, 1, 0, -1], [0, 1, 0, 0],
    [0, 0, 1, -1], [0, 0, 1, 0], [0, 1, 1, -2], [0, 1, 1, -1],
    [1, -1, -1, 1], [1, -1, -1, 2], [1, 0, -1, 0], [1, 0, -1, 1],
    [1, -1, 0, 0], [1, -1, 0, 1], [1, 0, 0, -1], [1, 0, 0, 0],
], dtype=np.float32)  # [16 gates, 4] -> c0,c1,c2,c3 = GATE_M.T @ softmax(w)


# ---------------------------------------------------------------------------
# static schedule: the merge-tree op list (DFS order keeps live tiles small)
# ---------------------------------------------------------------------------
def _build_schedule():
    """Each mix op: dict(level, key, lanes, base, node[lanes], kern[lanes]).
    L0 ops gather their own leaves; level l>=1 ops read T_{l-1}[2k],[2k+1]."""
    ops = []

    def emit(l, key):
        if l == 0:
            lanes = np.arange(128)
            ops.append(dict(level=0, key=key, lanes=128, base=0,
                            node=key + 32 * (lanes >> 6), kern=lanes & 63))
            return
        emit(l - 1, 2 * key)
        emit(l - 1, 2 * key + 1)
        lanes = np.arange(128)
        nbits_out = 6 - l
        ops.append(dict(level=l, key=key, lanes=128, base=0,
                        node=((lanes >> 6) << (nbits_out - 1)) + key,
                        kern=lanes & 63))

    emit(4, 0)
    emit(4, 1)
    # L5: one full op; node i5 = lane>>6 (a DMA then realigns the top half
    # to a base-0 tile for L6's equal-base inputs)
    lanes = np.arange(128)
    ops.append(dict(level=5, key=0, lanes=128, base=0,
                    node=lanes >> 6, kern=lanes & 63))
    lanes = np.arange(64)
    ops.append(dict(level=6, key=0, lanes=64, base=0,
                    node=np.zeros(64, np.int64), kern=lanes))
    return ops


_SCHED = _build_schedule()
_NMIX = len(_SCHED)          # 64
_NCOLS = 4 * _NMIX + 4       # + final root-bias column block


def _softmax_f32(w):
    w = w.astype(np.float64)
    m = w.max(-1, keepdims=True)
    e = np.exp(w - m)
    return e / e.sum(-1, keepdims=True)


def _coef_tables(ws):
    """ws = [w0..w6]. Returns coef matrix [128, _NCOLS] f32 with per-op scalar
    columns (alpha, beta, gamma, 0) and the final root-bias column
    (value to ADD on host: -T_root)."""
    cs = []
    for wl in ws:
        p = _softmax_f32(wl)                      # [s, K, 16] f64
        cs.append(np.einsum('skg,gj->skj', p, GATE_M.astype(np.float64)))
    # bias chain: T[l][node, kern] = delivered - true value at level-l output
    T = [None] * 7
    for l in range(7):
        c0, c1, c2, c3 = (cs[l][:, :, j] for j in range(4))
        if l == 0:
            Ta = np.zeros_like(c0)
            Tb = np.zeros_like(c0)
        else:
            Ta = T[l - 1][0::2]
            Tb = T[l - 1][1::2]
        T[l] = c2 * Tb + c1 * Ta - c3 * Ta * Tb - c0
    coef = np.zeros((128, _NCOLS), dtype=np.float64)
    for i, op in enumerate(_SCHED):
        l, node, kern = op['level'], op['node'], op['kern']
        rows = op['base'] + np.arange(op['lanes'])
        c = cs[l][node, kern]                     # [lanes, 4] = c0,c1,c2,c3
        if l == 0:
            Ta = np.zeros(op['lanes'])
            Tb = np.zeros(op['lanes'])
        else:
            Ta = T[l - 1][2 * node, kern]
            Tb = T[l - 1][2 * node + 1, kern]
        coef[rows, 4 * i + 0] = c[:, 3]                      # alpha = c3
        coef[rows, 4 * i + 1] = c[:, 2] - c[:, 3] * Ta       # beta
        coef[rows, 4 * i + 2] = c[:, 1] - c[:, 3] * Tb       # gamma
    coef[0:64, 4 * _NMIX] = -T[6][0, :]                      # final add
    return coef.astype(np.float32)


def _offset_tables(idx_a, idx_b):
    """Indirect-gather element-offset tables [128, 64] i32: col = 2*t + side.
    Offset = (c*9 + ha*3 + wa) * ROWE into the fp16 crop table."""
    offs = np.zeros((128, 64), dtype=np.int64)
    for op in _SCHED:
        if op['level'] != 0:
            continue
        t = op['key']
        for side, idx in ((0, idx_a), (1, idx_b)):
            ha = idx[op['kern'], 0, op['node'], 0].astype(np.int64)
            wa = idx[op['kern'], 0, op['node'], 1].astype(np.int64)
            ca = idx[op['kern'], 0, op['node'], 2].astype(np.int64)
            offs[:, 2 * t + side] = (ca * 9 + ha * 3 + wa) * ROWE
    return offs.astype(np.int32)


def _crop_table(xs):
    """xs: [C, H, W, B2] f32 b-interleaved slice -> XS [576, 1920] fp16."""
    XS = np.zeros((NROW, ROWE), dtype=np.float16)
    for ha in range(RF):
        for wa in range(RF):
            rows = np.arange(C) * 9 + ha * 3 + wa
            XS[rows, :F] = xs[:, ha:ha + PW, wa:wa + PW, :].reshape(
                C, F).astype(np.float16)
    return XS


# ---------------------------------------------------------------------------
# numpy emulator (mirrors the device schedule incl. fp16 rounding)
# ---------------------------------------------------------------------------
def _emulate_core(XS, offs, coef):
    """XS: [576,1920] fp16; offs: [128, 64] i32. Returns [64, F] f32."""
    f16 = np.float16
    XSf = XS.reshape(-1)
    tiles = {}
    for i, op in enumerate(_SCHED):
        l, key, n, base = op['level'], op['key'], op['lanes'], op['base']
        rws = base + np.arange(n)
        al = coef[rws, 4 * i + 0][:, None].astype(np.float32)
        be = coef[rws, 4 * i + 1][:, None].astype(np.float32)
        ga = coef[rws, 4 * i + 2][:, None].astype(np.float32)
        if l == 0:
            a = np.stack([XSf[o:o + F] for o in offs[:, 2 * key]])
            b = np.stack([XSf[o:o + F] for o in offs[:, 2 * key + 1]])
            a = a.astype(np.float32)
            b = b.astype(np.float32)
        elif l < 5:
            a = tiles[(l - 1, 2 * key)].astype(np.float32)
            b = tiles[(l - 1, 2 * key + 1)].astype(np.float32)
        elif l == 5:
            a = tiles[(4, 0)].astype(np.float32)
            b = tiles[(4, 1)].astype(np.float32)
        else:
            a = tiles['T5'][0:64].astype(np.float32)
            b = tiles['T5'][64:128].astype(np.float32)
        p = f16(a * al + be).astype(np.float32)
        q = f16(b * p).astype(np.float32)
        u = f16(a * ga).astype(np.float32)
        r = f16(q + u)
        if l == 5:
            tiles['T5'] = r
        else:
            tiles[(l, key)] = r
    return tiles[(6, 0)].astype(np.float32)


# ---------------------------------------------------------------------------
# custom DVE ops: the whole mix in 2 fused DVE instructions, with
# hand-authored 2x_1p perf-mode uop programs (the stock lower() only emits
# the 1x program; without uops_2x a custom op runs at 1 elem/cycle/lane).
#   MIX_PQ : out = (Src0*C0 + C1) * Src1     [q = (a*alpha + beta) * b]
#   AFF_ADD: out =  Src0*C0 + Src1           [r = a*gamma + q]
# 2x program structure (cribbed from the stock TENSOR_TENSOR 2X_1PORT entry):
# crossbar lane 0 feeds the ALU chain, lanes 1..6 the delay regs d0..d5;
# the LO chain computes on slices 0..k-1 while HI operands ride the delay
# regs; the HI chain computes on slices k..2k-1 while the LO result is
# captured into d5; WR0_LO <- DELAY_5, WR0_HI <- ALU_OUT.
# ---------------------------------------------------------------------------
_CUSTOM_REG = {}


def _register_custom_ops():
    if _CUSTOM_REG:
        return _CUSTOM_REG
    import concourse.dve_ops as dve_ops
    from concourse.dve_ops import DveOp
    from concourse.dve_spec import Spec, Src0, Src1, C0, C1, lower
    from concourse.dve_uop import (
        DveOpSpec, UopConfig, UopDpConfig, InpSel, OutSel, OutPath,
        AluInp, DelayInp, AluOp, Trigger)

    KEEP = DelayInp.PREV_DELAY
    CAPT = DelayInp.PREV_ALU_OUT
    PA = AluInp.PREV_ALU_OUT
    PD = [AluInp.PREV_DELAY_0, AluInp.PREV_DELAY_1, AluInp.PREV_DELAY_2,
          AluInp.PREV_DELAY_3, AluInp.PREV_DELAY_4, AluInp.PREV_DELAY_5]

    def dp(op=AluOp.BYPASS, s0=PA, s1=PA, keep=(), capt5=False):
        delay = [KEEP if k in keep else DelayInp.PREV_ALU_OUT for k in range(7)]
        den = [1 if k in keep else 0 for k in range(7)]
        if capt5:
            delay[5] = CAPT
            den[5] = 1
        return UopDpConfig(op=op, alu_src0=s0, alu_src1=s1, delay=delay,
                           alu_out_enable=1, delay_enable=den)

    def mk2x(lanes, stages, n_in):
        """lanes: 8 InpSel; stages: list of per-slice dp configs."""
        inp_en = [1 if lanes[k] != InpSel.ZERO or k == 0 else 0
                  for k in range(8)]
        return UopConfig(
            inp=lanes, inp_enable=inp_en,
            out={OutPath.WR0_LO: OutSel.DELAY_5,
                 OutPath.WR0_HI: OutSel.ALU_OUT,
                 OutPath.WR1_LO: OutSel.ALU_OUT,
                 OutPath.WR1_HI: OutSel.ALU_OUT},
            out_enable={OutPath.WR0_LO: 1, OutPath.WR0_HI: 1,
                        OutPath.WR1_LO: 0, OutPath.WR1_HI: 0},
            require_inp0=1, require_inp1=1,
            trigger=(Trigger.SRC_TENSOR_DONE, Trigger.NONE, Trigger.NONE),
            datapath_config=stages)

    M, A, BP = AluOp.MULTIPLY, AluOp.ADD, AluOp.BYPASS
    Z = InpSel.ZERO

    # ---- MIX_PQ: lanes: alu=SRC_0, d0=C0, d1=C1, d2=SRC_1, d3=SRC_0_HI,
    #      d4=SRC_1_HI
    mixpq_2x = mk2x(
        [InpSel.SRC_0, InpSel.CONST_0, InpSel.CONST_1, InpSel.SRC_1,
         InpSel.SRC_0_HI, InpSel.SRC_1_HI, Z, Z],
        [
            dp(M, PA, PD[0], keep=(0, 1, 2, 3, 4)),       # m_lo = a*C0
            dp(A, PA, PD[1], keep=(0, 1, 2, 3, 4)),       # a_lo = m_lo+C1
            dp(M, PA, PD[2], keep=(0, 1, 3, 4)),          # q_lo = a_lo*b
            dp(M, PD[3], PD[0], keep=(0, 1, 4), capt5=True),  # m_hi; d5<-q_lo
            dp(A, PA, PD[1], keep=(4, 5)),                # a_hi = m_hi+C1
            dp(M, PA, PD[4], keep=(5,)),                  # q_hi = a_hi*b_hi
            dp(BP, PA, PA, keep=(5,)),
            dp(BP, PA, PA, keep=(5,)),
        ], 6)

    # ---- AFF_ADD: lanes: alu=SRC_0, d0=C0, d1=SRC_1, d2=SRC_0_HI,
    #      d3=SRC_1_HI
    affadd_2x = mk2x(
        [InpSel.SRC_0, InpSel.CONST_0, InpSel.SRC_1, InpSel.SRC_0_HI,
         InpSel.SRC_1_HI, Z, Z, Z],
        [
            dp(M, PA, PD[0], keep=(0, 1, 2, 3)),          # m_lo = a*C0
            dp(A, PA, PD[1], keep=(0, 2, 3)),             # r_lo = m_lo+q
            dp(M, PD[2], PD[0], keep=(3,), capt5=True),   # m_hi; d5<-r_lo
            dp(A, PA, PD[3], keep=(5,)),                  # r_hi = m_hi+q_hi
            dp(BP, PA, PA, keep=(5,)),
            dp(BP, PA, PA, keep=(5,)),
            dp(BP, PA, PA, keep=(5,)),
            dp(BP, PA, PA, keep=(5,)),
        ], 5)

    defs = [
        ("ANT_LC_MIX_PQ",
         Spec(body=(Src0 * C0 + C1) * Src1,
              reference=lambda in0, in1, s0, s1, imm2:
              ((in0.astype(np.float32) * s0 + s1) * in1)),
         mixpq_2x),
        ("ANT_LC_AFF_ADD",
         Spec(body=Src0 * C0 + Src1,
              reference=lambda in0, in1, s0, s1, imm2:
              (in0.astype(np.float32) * s0 + in1)),
         affadd_2x),
    ]
    ver = "v3"
    for name, spec, u2x in defs:
        row = 1 + len(dve_ops.OPS)
        assert row < 0x20
        uops_1x = lower(spec, ver=ver)
        compiled = DveOpSpec(name=name, opcode=row, uops=uops_1x,
                             uops_2x=[u2x], rd1_en=True, perf_max=1)
        compiled.validate(ver)
        op = DveOp(name, spec, subdim=False,
                   uops_sha={ver: compiled.sha(ver)})
        dve_ops.OPS.append(op)
        dve_ops.CUSTOM_DVE_SPECS[name] = spec
        dve_ops._SUB_OPCODE_FOR_NAME[name] = row
        dve_ops._COMPILE_CACHE[(name, ver)] = compiled
        _CUSTOM_REG[name] = (op, row)
    return _CUSTOM_REG


def _emit_custom(nc, name, out, in0, in1, s0, s1=0.0, perf_max=1):
    """Emit one custom-DVE instruction (like bass Vector._custom_dve but
    with perf_max set so the engine may use the 2x_1p program)."""
    import concourse.bass_isa as bass_isa
    import concourse.mybir as mybir
    from concourse.dve_ops import get_dve_sub_opcode
    vec = nc.vector
    if name not in nc.m.ant_custom_dve_ops:
        nc.m.ant_custom_dve_ops = sorted({*nc.m.ant_custom_dve_ops, name})
    shape = bass_isa.CustomDveShape.TTSS
    isa_opcode = nc.isa.Opcode[
        f"NEURON_ISA_TPB_OPCODE_CUSTOM_DVE_ANT_{shape.slot()}"].value

    def lo_scalar(v):
        if isinstance(v, (int, float)):
            return mybir.ImmediateValue(dtype=mybir.dt.float32, value=float(v))
        return vec.lower_ap(v, for_isa=True)

    ins = [vec.lower_ap(in0, for_isa=True, opt=True),
           vec.lower_ap(in1, for_isa=True, opt=True),
           lo_scalar(s0), lo_scalar(s1)]
    outs = [vec.lower_ap(out, for_isa=True, opt=True)]
    return vec.add_instruction(
        bass_isa.InstCustomDveAnt(
            name=nc.get_next_instruction_name(),
            op_name=name, rd1_en=True, subdim=0, imm2=0.0, shape=shape,
            row=get_dve_sub_opcode(name), isa_opcode=isa_opcode,
            perf_max=perf_max, ins=ins, outs=outs))


# ---------------------------------------------------------------------------
# Bass program (built once, cached)
# ---------------------------------------------------------------------------
_BASS_CACHE = {}


def _build_bass():
    if 'nc' in _BASS_CACHE:
        return _BASS_CACHE['nc']
    import concourse.bass as bass
    import concourse.mybir as mybir
    import concourse.tile as tile
    import concourse.bacc as bacc

    _register_custom_ops()
    f32 = mybir.dt.float32
    f16 = mybir.dt.float16
    nc = bacc.Bacc("TRN2", target_bir_lowering=False, debug=False,
                   num_devices=NCORES)
    nxs = NROW * ROWE
    xs_d = nc.dram_tensor("xs", [nxs, 1], f16, kind="ExternalInput").ap()
    offs_d = nc.dram_tensor("offs", [128, 64], mybir.dt.int32,
                            kind="ExternalInput").ap()
    coef_d = nc.dram_tensor("coef", [128, _NCOLS], f32,
                            kind="ExternalInput").ap()
    out_d = nc.dram_tensor("out", [64, F], f16, kind="ExternalOutput").ap()

    AL = mybir.AluOpType
    ACTF = mybir.ActivationFunctionType

    with tile.TileContext(nc) as tc:
        with (
            tc.tile_pool(name="const", bufs=1) as pc,
            tc.tile_pool(name="ab", bufs=10) as pab,
            tc.tile_pool(name="lvl", bufs=3) as plv,
            tc.tile_pool(name="t0p", bufs=3) as pt0,
            tc.tile_pool(name="tmp", bufs=8) as ptmp,
            tc.tile_pool(name="fin", bufs=1) as pfin,
        ):
            offs_t = pc.tile([128, 64], mybir.dt.int32, tag="offs",
                             name="offs_t")
            nc.sync.dma_start(offs_t[:], offs_d[:])
            coef_t = pc.tile([128, _NCOLS], f32, tag="coef", name="coef_t")
            nc.sync.dma_start(coef_t[:], coef_d[:])
            warm_t = pc.tile([1, 8], f32, tag="warm", name="warm_t")
            nc.scalar.activation(warm_t[:], coef_t[0:1, 0:8],
                                 ACTF.Identity, bias=0.0, scale=1.0)

            tiles = {}
            gtiles = {}
            for i, op in enumerate(_SCHED):
                l, key, n, base = op['level'], op['key'], op['lanes'], op['base']
                sl = slice(base, base + n)
                al = coef_t[sl, 4 * i + 0:4 * i + 1]
                be = coef_t[sl, 4 * i + 1:4 * i + 2]
                ga = coef_t[sl, 4 * i + 2:4 * i + 3]
                if l == 0:
                    if NCOL_GATHER > 1:
                        gk = (key // NCOL_GATHER) * NCOL_GATHER
                        if key == gk:
                            g_t = pab.tile([128, 2 * NCOL_GATHER * F], f16,
                                           tag="AB", name="ab_t")
                            g_ap = g_t[:].rearrange(
                                "p (j e) -> p j e", j=2 * NCOL_GATHER, e=F)
                            nc.gpsimd.indirect_dma_start(
                                out=g_ap, out_offset=None, in_=xs_d[:],
                                in_offset=bass.IndirectOffsetOnAxis(
                                    ap=offs_t[:, 2 * gk:
                                              2 * (gk + NCOL_GATHER)],
                                    axis=0))
                            for kk in range(gk, gk + NCOL_GATHER):
                                gtiles[kk] = g_t
                        g_t = gtiles[key]
                        half = key - gk
                        a_ap = g_t[:, 2 * half * F:(2 * half + 1) * F]
                        b_ap = g_t[:, (2 * half + 1) * F:(2 * half + 2) * F]
                    else:
                        a_t = pab.tile([128, F], f16, tag="A", name="at")
                        b_t = pab.tile([128, F], f16, tag="B", name="bt")
                        halves = ((0, 900), (900, 900)) if i == 0 \
                            else ((0, F),)
                        for off, ln in halves:
                            for side, dst in ((0, a_t), (1, b_t)):
                                nc.gpsimd.indirect_dma_start(
                                    out=dst[:, off:off + ln],
                                    out_offset=None, in_=xs_d[:],
                                    in_offset=bass.IndirectOffsetOnAxis(
                                        ap=offs_t[:, 2 * key + side:
                                                  2 * key + side + 1],
                                        axis=0),
                                    element_offset=off)
                        a_ap, b_ap = a_t[:, :], b_t[:, :]
                elif l < 5:
                    a_ap = tiles[(l - 1, 2 * key)][:, :]
                    b_ap = tiles[(l - 1, 2 * key + 1)][:, :]
                elif l == 5:
                    a_ap = tiles[(4, 0)][:, :]
                    b_ap = tiles[(4, 1)][:, :]
                else:
                    a_ap = tiles['T5'][0:64, :]
                    b_ap = tiles['T5b'][:, :]

                # q = (a*alpha + beta) * b   [fused custom DVE op, 2x]
                # (offload experiments regressed: ops with upstream compute
                # deps stall the in-order Pool engine and block later gather
                # descgens -- keep Pool gather-only, all mixes on DVE)
                # h-split the first mix (faster ramp) and the L5/L6 tail
                # (overlaps the T5b realign DMA / output store with compute)
                hsp = ((0, 900), (900, 900)) if (i == 0 or l >= 5) \
                    else ((0, F),)
                p_t = ptmp.tile([n, F], f16, tag="p", name="p")
                for off, ln in hsp:
                    fs = slice(off, off + ln)
                    _emit_custom(nc, "ANT_LC_MIX_PQ", p_t[:, fs],
                                 a_ap[:, fs], b_ap[:, fs], al, be)
                # output tile
                if l == 5:
                    r_t = pfin.tile([128, F], f16, tag="T5", name="t5")
                    tiles['T5'] = r_t
                elif l == 6:
                    r_t = pfin.tile([64, F], f16, tag="T6", name="t6")
                else:
                    pool = pt0 if l == 0 else plv
                    r_t = pool.tile([128, F], f16, tag=f"T{l}",
                                    name=f"t{l}_{key}")
                    tiles[(l, key)] = r_t

                if l == 5:
                    t5b = pfin.tile([64, F], f16, tag="T5b", name="t5b")
                    tiles['T5b'] = t5b
                for off, ln in hsp:
                    fs = slice(off, off + ln)
                    _emit_custom(nc, "ANT_LC_AFF_ADD", r_t[:, fs],
                                 a_ap[:, fs], p_t[:, fs], ga)
                    if l == 5:
                        nc.sync.dma_start(tiles['T5b'][:, fs],
                                          r_t[64:128, fs])
                    elif l == 6:
                        nc.sync.dma_start(out_d[:, fs], r_t[:, fs])
    nc.compile()
    _BASS_CACHE['nc'] = nc
    return nc


def _prep_inputs(x, idx_a, idx_b, ws):
    coef = _coef_tables(ws)
    offs = _offset_tables(idx_a, idx_b)
    x = np.ascontiguousarray(x, dtype=np.float32)
    in_maps = []
    for core in range(NCORES):
        xs = x[B2 * core:B2 * core + B2].transpose(1, 2, 3, 0)  # [C,H,W,B2]
        in_maps.append({"xs": _crop_table(xs).reshape(-1, 1),
                        "offs": offs, "coef": coef})
    return in_maps


def _assemble(core_outs, gamma):
    """core_outs: list of [64, F=(hh,ww,b)]; gamma [64] -> [16,64,900,1]."""
    full = np.stack([np.asarray(o, dtype=np.float32) for o in core_outs])
    full = full + gamma.astype(np.float32)[None, :, None]
    full = full.reshape(NCORES, K, P, B2)           # [core, k, p, b_local]
    full = full.transpose(0, 3, 1, 2).reshape(B, K, P, 1)
    return np.ascontiguousarray(full.astype(np.float32))


def kernel(x, idx_a, idx_b, w0, w1, w2, w3, w4, w5, w6):
    ws = [np.asarray(w, dtype=np.float32) for w in
          (w0, w1, w2, w3, w4, w5, w6)]
    x = np.asarray(x, dtype=np.float32)
    idx_a = np.asarray(idx_a, dtype=np.int32)
    idx_b = np.asarray(idx_b, dtype=np.int32)
    in_maps = _prep_inputs(x, idx_a, idx_b, ws)
    nc = _build_bass()
    from concourse.bass_utils import run_bass_kernel_spmd
    res = run_bass_kernel_spmd(nc, in_maps, core_ids=list(range(NCORES)))
    gamma = in_maps[0]["coef"][0:64, 4 * _NMIX]
    return _assemble([r["out"] for r in res.results], gamma)


def kernel_emulate(x, idx_a, idx_b, w0, w1, w2, w3, w4, w5, w6):
    """Pure-numpy emulation of the exact device schedule (debug aid)."""
    ws = [np.asarray(w, dtype=np.float32) for w in
          (w0, w1, w2, w3, w4, w5, w6)]
    in_maps = _prep_inputs(np.asarray(x, np.float32),
                           np.asarray(idx_a, np.int32),
                           np.asarray(idx_b, np.int32), ws)
    outs = [_emulate_core(m["xs"].reshape(NROW, ROWE), m["offs"], m["coef"])
            for m in in_maps]
    return _assemble(outs, in_maps[0]["coef"][0:64, 4 * _NMIX])


# revision 24
# speedup vs baseline: 1.0116x; 1.0116x over previous
"""Trainium2 Bass kernel for nn_LogicConv3d (differentiable-logic conv tree).

Problem (hardcoded): x [16,64,32,32] f32; idx_a/idx_b [64,900,64,3] i32;
w0..w6 [s,64,16] f32 (s = 64,32,16,8,4,2,1). Output [16,64,900,1] f32.

Math: per (kernel k, window p): gather 64 (a,b) leaf pairs from x, blend each
pair with soft-gate coefficients (softmax(w) @ GATE_M), then 6 more pairwise
tree levels.  mix(a,b) = c0 + c1*a + c2*b + c3*a*b.

v3 design (fp16 end-to-end, DVE 2x/4x perf modes):
 - F-sharding: core i handles batches (2i, 2i+1); pure SPMD across 8 cores.
 - Host builds a 576-row fp16 crop table XS[(c,ha,wa), 1920]: row = the
   30x30x2 (h,w,b-interleaved) crop of channel c at shift (ha,wa), compact in
   the first 1800 elements.  Leaf gathers are indirect DMAs with per-lane
   element offsets row*1920, fetching 1800 contiguous fp16 -> operands are
   step-1 fp16, which unlocks DVE packed modes.
 - scalar_tensor_tensor has NO DVE perf modes (1x only) so the mix avoids it:
     p = a*alpha + beta   (tensor_scalar 4x on DVE, or ACT activation)
     q = b * p            (tensor_tensor mult, 2x on DVE; some on Pool)
     u = a*gamma          (tensor_scalar / ACT)
     out = q + u          (tensor_tensor add, 2x on DVE)
   alpha = c3, beta = c2 - c3*Ta, gamma = c1 - c3*Tb where Ta/Tb are the
   children's additive-bias chain (bias folding; all multiplicative, safe).
   The per-node bias T = c2*Tb + c1*Ta - c3*Ta*Tb - c0 propagates on host in
   f64; the root bias is subtracted on host.
"""
import numpy as np

B, C, H, W = 16, 64, 32, 32
K = 64
RF = 3
DEPTH = 6
S = 64
PW = 30            # windows per axis
P = PW * PW        # 900
NCORES = 8
B2 = 2             # batches per core
F = P * B2         # free size (windows x batches) = 1800
NROW = C * RF * RF          # 576 crop-table rows
ROWE = 1920                 # crop-table row stride (elements)

# static engine assignment (tuned against the HW trace; see LP in notes):
#  - all p-ops on ACT
#  - q-TT: Pool for i%16 in [0,7), else DVE
#  - u+add: fused stt on DVE, except i%4==0 mixes use CCE-DMA add with a
#    separate u tile (u on ACT, or DVE-ts when i%16==0)
def _q_on_pool(i):
    return i % 9 < 2          # ~14 q-TT ops on Pool


def _use_cce(i, level):
    return i % 8 < 3 and level < 6   # 24 mixes: add via CCE-DMA


def _u_on_act(i):
    return True               # u for CCE mixes always on ACT


def _p_on_act(i):
    return i % 8 != 3         # 56 p-ops on ACT, 8 on DVE-ts


NCOL_GATHER = 1      # L0 ops batched per indirect gather DMA (1 = per-side)

GATE_M = np.array([
    [0, 0, 0, 0], [0, 0, 0, 1], [0, 1, 0, -1], [0, 1, 0, 0],
    [0, 0, 1, -1], [0, 0, 1, 0], [0, 1, 1, -2], [0, 1, 1, -1],
    [1, -1, -1, 1], [1, -1, -1, 2], [1, 0, -1, 0], [1, 0, -1, 1],
    [1, -1, 0, 0], [1, -1, 0, 1], [1, 0, 0, -1], [1, 0, 0, 0],
], dtype=np.float32)  # [16 gates, 4] -> c0,c1,c2,c3 = GATE_M.T @ softmax(w)


# ---------------------------------------------------------------------------
# static schedule: the merge-tree op list (DFS order keeps live tiles small)
# ---------------------------------------------------------------------------
def _build_schedule():
    """Each mix op: dict(level, key, lanes, base, node[lanes], kern[lanes]).
    L0 ops gather their own leaves; level l>=1 ops read T_{l-1}[2k],[2k+1]."""
    ops = []

    def emit(l, key):
        if l == 0:
            lanes = np.arange(128)
            ops.append(dict(level=0, key=key, lanes=128, base=0,
                            node=key + 32 * (lanes >> 6), kern=lanes & 63))
            return
        emit(l - 1, 2 * key)
        emit(l - 1, 2 * key + 1)
        lanes = np.arange(128)
        nbits_out = 6 - l
        ops.append(dict(level=l, key=key, lanes=128, base=0,
                        node=((lanes >> 6) << (nbits_out - 1)) + key,
                        kern=lanes & 63))

    emit(4, 0)
    emit(4, 1)
    # L5: one full op; node i5 = lane>>6 (a DMA then realigns the top half
    # to a base-0 tile for L6's equal-base inputs)
    lanes = np.arange(128)
    ops.append(dict(level=5, key=0, lanes=128, base=0,
                    node=lanes >> 6, kern=lanes & 63))
    lanes = np.arange(64)
    ops.append(dict(level=6, key=0, lanes=64, base=0,
                    node=np.zeros(64, np.int64), kern=lanes))
    return ops


_SCHED = _build_schedule()
_NMIX = len(_SCHED)          # 64
_NCOLS = 4 * _NMIX + 4       # + final root-bias column block


def _softmax_f32(w):
    w = w.astype(np.float64)
    m = w.max(-1, keepdims=True)
    e = np.exp(w - m)
    return e / e.sum(-1, keepdims=True)


def _coef_tables(ws):
    """ws = [w0..w6]. Returns coef matrix [128, _NCOLS] f32 with per-op scalar
    columns (alpha, beta, gamma, 0) and the final root-bias column
    (value to ADD on host: -T_root)."""
    cs = []
    for wl in ws:
        p = _softmax_f32(wl)                      # [s, K, 16] f64
        cs.append(np.einsum('skg,gj->skj', p, GATE_M.astype(np.float64)))
    # bias chain: T[l][node, kern] = delivered - true value at level-l output
    T = [None] * 7
    for l in range(7):
        c0, c1, c2, c3 = (cs[l][:, :, j] for j in range(4))
        if l == 0:
            Ta = np.zeros_like(c0)
            Tb = np.zeros_like(c0)
        else:
            Ta = T[l - 1][0::2]
            Tb = T[l - 1][1::2]
        T[l] = c2 * Tb + c1 * Ta - c3 * Ta * Tb - c0
    coef = np.zeros((128, _NCOLS), dtype=np.float64)
    for i, op in enumerate(_SCHED):
        l, node, kern = op['level'], op['node'], op['kern']
        rows = op['base'] + np.arange(op['lanes'])
        c = cs[l][node, kern]                     # [lanes, 4] = c0,c1,c2,c3
        if l == 0:
            Ta = np.zeros(op['lanes'])
            Tb = np.zeros(op['lanes'])
        else:
            Ta = T[l - 1][2 * node, kern]
            Tb = T[l - 1][2 * node + 1, kern]
        coef[rows, 4 * i + 0] = c[:, 3]                      # alpha = c3
        coef[rows, 4 * i + 1] = c[:, 2] - c[:, 3] * Ta       # beta
        coef[rows, 4 * i + 2] = c[:, 1] - c[:, 3] * Tb       # gamma
    coef[0:64, 4 * _NMIX] = -T[6][0, :]                      # final add
    return coef.astype(np.float32)


def _offset_tables(idx_a, idx_b):
    """Indirect-gather element-offset tables [128, 64] i32: col = 2*t + side.
    Offset = (c*9 + ha*3 + wa) * ROWE into the fp16 crop table."""
    offs = np.zeros((128, 64), dtype=np.int64)
    for op in _SCHED:
        if op['level'] != 0:
            continue
        t = op['key']
        for side, idx in ((0, idx_a), (1, idx_b)):
            ha = idx[op['kern'], 0, op['node'], 0].astype(np.int64)
            wa = idx[op['kern'], 0, op['node'], 1].astype(np.int64)
            ca = idx[op['kern'], 0, op['node'], 2].astype(np.int64)
            offs[:, 2 * t + side] = (ca * 9 + ha * 3 + wa) * ROWE
    return offs.astype(np.int32)


def _crop_table(xs):
    """xs: [C, H, W, B2] f32 b-interleaved slice -> XS [576, 1920] fp16."""
    XS = np.zeros((NROW, ROWE), dtype=np.float16)
    for ha in range(RF):
        for wa in range(RF):
            rows = np.arange(C) * 9 + ha * 3 + wa
            XS[rows, :F] = xs[:, ha:ha + PW, wa:wa + PW, :].reshape(
                C, F).astype(np.float16)
    return XS


# ---------------------------------------------------------------------------
# numpy emulator (mirrors the device schedule incl. fp16 rounding)
# ---------------------------------------------------------------------------
def _emulate_core(XS, offs, coef):
    """XS: [576,1920] fp16; offs: [128, 64] i32. Returns [64, F] f32."""
    f16 = np.float16
    XSf = XS.reshape(-1)
    tiles = {}
    for i, op in enumerate(_SCHED):
        l, key, n, base = op['level'], op['key'], op['lanes'], op['base']
        rws = base + np.arange(n)
        al = coef[rws, 4 * i + 0][:, None].astype(np.float32)
        be = coef[rws, 4 * i + 1][:, None].astype(np.float32)
        ga = coef[rws, 4 * i + 2][:, None].astype(np.float32)
        if l == 0:
            a = np.stack([XSf[o:o + F] for o in offs[:, 2 * key]])
            b = np.stack([XSf[o:o + F] for o in offs[:, 2 * key + 1]])
            a = a.astype(np.float32)
            b = b.astype(np.float32)
        elif l < 5:
            a = tiles[(l - 1, 2 * key)].astype(np.float32)
            b = tiles[(l - 1, 2 * key + 1)].astype(np.float32)
        elif l == 5:
            a = tiles[(4, 0)].astype(np.float32)
            b = tiles[(4, 1)].astype(np.float32)
        else:
            a = tiles['T5'][0:64].astype(np.float32)
            b = tiles['T5'][64:128].astype(np.float32)
        p = f16(a * al + be).astype(np.float32)
        q = f16(b * p).astype(np.float32)
        u = f16(a * ga).astype(np.float32)
        r = f16(q + u)
        if l == 5:
            tiles['T5'] = r
        else:
            tiles[(l, key)] = r
    return tiles[(6, 0)].astype(np.float32)


# ---------------------------------------------------------------------------
# custom DVE ops: the whole mix in 2 fused DVE instructions, with
# hand-authored 2x_1p perf-mode uop programs (the stock lower() only emits
# the 1x program; without uops_2x a custom op runs at 1 elem/cycle/lane).
#   MIX_PQ : out = (Src0*C0 + C1) * Src1     [q = (a*alpha + beta) * b]
#   AFF_ADD: out =  Src0*C0 + Src1           [r = a*gamma + q]
# 2x program structure (cribbed from the stock TENSOR_TENSOR 2X_1PORT entry):
# crossbar lane 0 feeds the ALU chain, lanes 1..6 the delay regs d0..d5;
# the LO chain computes on slices 0..k-1 while HI operands ride the delay
# regs; the HI chain computes on slices k..2k-1 while the LO result is
# captured into d5; WR0_LO <- DELAY_5, WR0_HI <- ALU_OUT.
# ---------------------------------------------------------------------------
_CUSTOM_REG = {}


def _register_custom_ops():
    if _CUSTOM_REG:
        return _CUSTOM_REG
    import concourse.dve_ops as dve_ops
    from concourse.dve_ops import DveOp
    from concourse.dve_spec import Spec, Src0, Src1, C0, C1, lower
    from concourse.dve_uop import (
        DveOpSpec, UopConfig, UopDpConfig, InpSel, OutSel, OutPath,
        AluInp, DelayInp, AluOp, Trigger)

    KEEP = DelayInp.PREV_DELAY
    CAPT = DelayInp.PREV_ALU_OUT
    PA = AluInp.PREV_ALU_OUT
    PD = [AluInp.PREV_DELAY_0, AluInp.PREV_DELAY_1, AluInp.PREV_DELAY_2,
          AluInp.PREV_DELAY_3, AluInp.PREV_DELAY_4, AluInp.PREV_DELAY_5]

    def dp(op=AluOp.BYPASS, s0=PA, s1=PA, keep=(), capt5=False):
        delay = [KEEP if k in keep else DelayInp.PREV_ALU_OUT for k in range(7)]
        den = [1 if k in keep else 0 for k in range(7)]
        if capt5:
            delay[5] = CAPT
            den[5] = 1
        return UopDpConfig(op=op, alu_src0=s0, alu_src1=s1, delay=delay,
                           alu_out_enable=1, delay_enable=den)

    def mk2x(lanes, stages, n_in):
        """lanes: 8 InpSel; stages: list of per-slice dp configs."""
        inp_en = [1 if lanes[k] != InpSel.ZERO or k == 0 else 0
                  for k in range(8)]
        return UopConfig(
            inp=lanes, inp_enable=inp_en,
            out={OutPath.WR0_LO: OutSel.DELAY_5,
                 OutPath.WR0_HI: OutSel.ALU_OUT,
                 OutPath.WR1_LO: OutSel.ALU_OUT,
                 OutPath.WR1_HI: OutSel.ALU_OUT},
            out_enable={OutPath.WR0_LO: 1, OutPath.WR0_HI: 1,
                        OutPath.WR1_LO: 0, OutPath.WR1_HI: 0},
            require_inp0=1, require_inp1=1,
            trigger=(Trigger.SRC_TENSOR_DONE, Trigger.NONE, Trigger.NONE),
            datapath_config=stages)

    M, A, BP = AluOp.MULTIPLY, AluOp.ADD, AluOp.BYPASS
    Z = InpSel.ZERO

    # ---- MIX_PQ: lanes: alu=SRC_0, d0=C0, d1=C1, d2=SRC_1, d3=SRC_0_HI,
    #      d4=SRC_1_HI
    mixpq_2x = mk2x(
        [InpSel.SRC_0, InpSel.CONST_0, InpSel.CONST_1, InpSel.SRC_1,
         InpSel.SRC_0_HI, InpSel.SRC_1_HI, Z, Z],
        [
            dp(M, PA, PD[0], keep=(0, 1, 2, 3, 4)),       # m_lo = a*C0
            dp(A, PA, PD[1], keep=(0, 1, 2, 3, 4)),       # a_lo = m_lo+C1
            dp(M, PA, PD[2], keep=(0, 1, 3, 4)),          # q_lo = a_lo*b
            dp(M, PD[3], PD[0], keep=(0, 1, 4), capt5=True),  # m_hi; d5<-q_lo
            dp(A, PA, PD[1], keep=(4, 5)),                # a_hi = m_hi+C1
            dp(M, PA, PD[4], keep=(5,)),                  # q_hi = a_hi*b_hi
            dp(BP, PA, PA, keep=(5,)),
            dp(BP, PA, PA, keep=(5,)),
        ], 6)

    # ---- AFF_ADD: lanes: alu=SRC_0, d0=C0, d1=SRC_1, d2=SRC_0_HI,
    #      d3=SRC_1_HI
    affadd_2x = mk2x(
        [InpSel.SRC_0, InpSel.CONST_0, InpSel.SRC_1, InpSel.SRC_0_HI,
         InpSel.SRC_1_HI, Z, Z, Z],
        [
            dp(M, PA, PD[0], keep=(0, 1, 2, 3)),          # m_lo = a*C0
            dp(A, PA, PD[1], keep=(0, 2, 3)),             # r_lo = m_lo+q
            dp(M, PD[2], PD[0], keep=(3,), capt5=True),   # m_hi; d5<-r_lo
            dp(A, PA, PD[3], keep=(5,)),                  # r_hi = m_hi+q_hi
            dp(BP, PA, PA, keep=(5,)),
            dp(BP, PA, PA, keep=(5,)),
            dp(BP, PA, PA, keep=(5,)),
            dp(BP, PA, PA, keep=(5,)),
        ], 5)

    defs = [
        ("ANT_LC_MIX_PQ",
         Spec(body=(Src0 * C0 + C1) * Src1,
              reference=lambda in0, in1, s0, s1, imm2:
              ((in0.astype(np.float32) * s0 + s1) * in1)),
         mixpq_2x),
        ("ANT_LC_AFF_ADD",
         Spec(body=Src0 * C0 + Src1,
              reference=lambda in0, in1, s0, s1, imm2:
              (in0.astype(np.float32) * s0 + in1)),
         affadd_2x),
    ]
    ver = "v3"
    for name, spec, u2x in defs:
        row = 1 + len(dve_ops.OPS)
        assert row < 0x20
        uops_1x = lower(spec, ver=ver)
        compiled = DveOpSpec(name=name, opcode=row, uops=uops_1x,
                             uops_2x=[u2x], rd1_en=True, perf_max=1)
        compiled.validate(ver)
        op = DveOp(name, spec, subdim=False,
                   uops_sha={ver: compiled.sha(ver)})
        dve_ops.OPS.append(op)
        dve_ops.CUSTOM_DVE_SPECS[name] = spec
        dve_ops._SUB_OPCODE_FOR_NAME[name] = row
        dve_ops._COMPILE_CACHE[(name, ver)] = compiled
        _CUSTOM_REG[name] = (op, row)
    return _CUSTOM_REG


def _emit_custom(nc, name, out, in0, in1, s0, s1=0.0, perf_max=1):
    """Emit one custom-DVE instruction (like bass Vector._custom_dve but
    with perf_max set so the engine may use the 2x_1p program)."""
    import concourse.bass_isa as bass_isa
    import concourse.mybir as mybir
    from concourse.dve_ops import get_dve_sub_opcode
    vec = nc.vector
    if name not in nc.m.ant_custom_dve_ops:
        nc.m.ant_custom_dve_ops = sorted({*nc.m.ant_custom_dve_ops, name})
    shape = bass_isa.CustomDveShape.TTSS
    isa_opcode = nc.isa.Opcode[
        f"NEURON_ISA_TPB_OPCODE_CUSTOM_DVE_ANT_{shape.slot()}"].value

    def lo_scalar(v):
        if isinstance(v, (int, float)):
            return mybir.ImmediateValue(dtype=mybir.dt.float32, value=float(v))
        return vec.lower_ap(v, for_isa=True)

    ins = [vec.lower_ap(in0, for_isa=True, opt=True),
           vec.lower_ap(in1, for_isa=True, opt=True),
           lo_scalar(s0), lo_scalar(s1)]
    outs = [vec.lower_ap(out, for_isa=True, opt=True)]
    return vec.add_instruction(
        bass_isa.InstCustomDveAnt(
            name=nc.get_next_instruction_name(),
            op_name=name, rd1_en=True, subdim=0, imm2=0.0, shape=shape,
            row=get_dve_sub_opcode(name), isa_opcode=isa_opcode,
            perf_max=perf_max, ins=ins, outs=outs))


# ---------------------------------------------------------------------------
# Bass program (built once, cached)
# ---------------------------------------------------------------------------
_BASS_CACHE = {}


def _build_bass():
    if 'nc' in _BASS_CACHE:
        return _BASS_CACHE['nc']
    import concourse.bass as bass
    import concourse.mybir as mybir
    import concourse.tile as tile
    import concourse.bacc as bacc

    _register_custom_ops()
    f32 = mybir.dt.float32
    f16 = mybir.dt.float16
    nc = bacc.Bacc("TRN2", target_bir_lowering=False, debug=False,
                   num_devices=NCORES)
    nxs = NROW * ROWE
    xs_d = nc.dram_tensor("xs", [nxs, 1], f16, kind="ExternalInput").ap()
    offs_d = nc.dram_tensor("offs", [128, 64], mybir.dt.int32,
                            kind="ExternalInput").ap()
    coef_d = nc.dram_tensor("coef", [128, _NCOLS], f32,
                            kind="ExternalInput").ap()
    out_d = nc.dram_tensor("out", [64, F], f16, kind="ExternalOutput").ap()

    AL = mybir.AluOpType
    ACTF = mybir.ActivationFunctionType

    with tile.TileContext(nc) as tc:
        with (
            tc.tile_pool(name="const", bufs=1) as pc,
            tc.tile_pool(name="ab", bufs=10) as pab,
            tc.tile_pool(name="lvl", bufs=3) as plv,
            tc.tile_pool(name="t0p", bufs=3) as pt0,
            tc.tile_pool(name="tmp", bufs=8) as ptmp,
            tc.tile_pool(name="fin", bufs=1) as pfin,
        ):
            offs_t = pc.tile([128, 64], mybir.dt.int32, tag="offs",
                             name="offs_t")
            nc.sync.dma_start(offs_t[:], offs_d[:])
            coef_t = pc.tile([128, _NCOLS], f32, tag="coef", name="coef_t")
            nc.sync.dma_start(coef_t[:], coef_d[:])
            warm_t = pc.tile([1, 8], f32, tag="warm", name="warm_t")
            nc.scalar.activation(warm_t[:], coef_t[0:1, 0:8],
                                 ACTF.Identity, bias=0.0, scale=1.0)

            tiles = {}
            gtiles = {}
            for i, op in enumerate(_SCHED):
                l, key, n, base = op['level'], op['key'], op['lanes'], op['base']
                sl = slice(base, base + n)
                al = coef_t[sl, 4 * i + 0:4 * i + 1]
                be = coef_t[sl, 4 * i + 1:4 * i + 2]
                ga = coef_t[sl, 4 * i + 2:4 * i + 3]
                if l == 0:
                    if NCOL_GATHER > 1:
                        gk = (key // NCOL_GATHER) * NCOL_GATHER
                        if key == gk:
                            g_t = pab.tile([128, 2 * NCOL_GATHER * F], f16,
                                           tag="AB", name="ab_t")
                            g_ap = g_t[:].rearrange(
                                "p (j e) -> p j e", j=2 * NCOL_GATHER, e=F)
                            nc.gpsimd.indirect_dma_start(
                                out=g_ap, out_offset=None, in_=xs_d[:],
                                in_offset=bass.IndirectOffsetOnAxis(
                                    ap=offs_t[:, 2 * gk:
                                              2 * (gk + NCOL_GATHER)],
                                    axis=0))
                            for kk in range(gk, gk + NCOL_GATHER):
                                gtiles[kk] = g_t
                        g_t = gtiles[key]
                        half = key - gk
                        a_ap = g_t[:, 2 * half * F:(2 * half + 1) * F]
                        b_ap = g_t[:, (2 * half + 1) * F:(2 * half + 2) * F]
                    else:
                        a_t = pab.tile([128, F], f16, tag="A", name="at")
                        b_t = pab.tile([128, F], f16, tag="B", name="bt")
                        for side, dst in ((0, a_t), (1, b_t)):
                            nc.gpsimd.indirect_dma_start(
                                out=dst[:], out_offset=None, in_=xs_d[:],
                                in_offset=bass.IndirectOffsetOnAxis(
                                    ap=offs_t[:, 2 * key + side:
                                              2 * key + side + 1], axis=0))
                        a_ap, b_ap = a_t[:, :], b_t[:, :]
                elif l < 5:
                    a_ap = tiles[(l - 1, 2 * key)][:, :]
                    b_ap = tiles[(l - 1, 2 * key + 1)][:, :]
                elif l == 5:
                    a_ap = tiles[(4, 0)][:, :]
                    b_ap = tiles[(4, 1)][:, :]
                else:
                    a_ap = tiles['T5'][0:64, :]
                    b_ap = tiles['T5b'][:, :]

                # q = (a*alpha + beta) * b   [fused custom DVE op, 2x]
                # (offload experiments regressed: ops with upstream compute
                # deps stall the in-order Pool engine and block later gather
                # descgens -- keep Pool gather-only, all mixes on DVE)
                # (h-splitting the first mix / L5-L6 tail was tried and
                # regressed: extra per-op overhead + early-pipeline stalls
                # outweighed the ~1us tail-gap gain)
                hsp = ((0, 900), (900, 900)) if l == 6 else ((0, F),)
                p_t = ptmp.tile([n, F], f16, tag="p", name="p")
                _emit_custom(nc, "ANT_LC_MIX_PQ", p_t[:, :], a_ap, b_ap,
                             al, be)
                # output tile
                if l == 5:
                    r_t = pfin.tile([128, F], f16, tag="T5", name="t5")
                    tiles['T5'] = r_t
                elif l == 6:
                    r_t = pfin.tile([64, F], f16, tag="T6", name="t6")
                else:
                    pool = pt0 if l == 0 else plv
                    r_t = pool.tile([128, F], f16, tag=f"T{l}",
                                    name=f"t{l}_{key}")
                    tiles[(l, key)] = r_t

                if l == 5:
                    t5b = pfin.tile([64, F], f16, tag="T5b", name="t5b")
                    tiles['T5b'] = t5b
                for off, ln in hsp:
                    fs = slice(off, off + ln)
                    _emit_custom(nc, "ANT_LC_AFF_ADD", r_t[:, fs],
                                 a_ap[:, fs], p_t[:, fs], ga)
                    if l == 5:
                        nc.sync.dma_start(tiles['T5b'][:, fs],
                                          r_t[64:128, fs])
                    elif l == 6:
                        nc.sync.dma_start(out_d[:, fs], r_t[:, fs])
    nc.compile()
    _BASS_CACHE['nc'] = nc
    return nc


def _prep_inputs(x, idx_a, idx_b, ws):
    coef = _coef_tables(ws)
    offs = _offset_tables(idx_a, idx_b)
    x = np.ascontiguousarray(x, dtype=np.float32)
    in_maps = []
    for core in range(NCORES):
        xs = x[B2 * core:B2 * core + B2].transpose(1, 2, 3, 0)  # [C,H,W,B2]
        in_maps.append({"xs": _crop_table(xs).reshape(-1, 1),
                        "offs": offs, "coef": coef})
    return in_maps


def _assemble(core_outs, gamma):
    """core_outs: list of [64, F=(hh,ww,b)]; gamma [64] -> [16,64,900,1]."""
    full = np.stack([np.asarray(o, dtype=np.float32) for o in core_outs])
    full = full + gamma.astype(np.float32)[None, :, None]
    full = full.reshape(NCORES, K, P, B2)           # [core, k, p, b_local]
    full = full.transpose(0, 3, 1, 2).reshape(B, K, P, 1)
    return np.ascontiguousarray(full.astype(np.float32))


def kernel(x, idx_a, idx_b, w0, w1, w2, w3, w4, w5, w6):
    ws = [np.asarray(w, dtype=np.float32) for w in
          (w0, w1, w2, w3, w4, w5, w6)]
    x = np.asarray(x, dtype=np.float32)
    idx_a = np.asarray(idx_a, dtype=np.int32)
    idx_b = np.asarray(idx_b, dtype=np.int32)
    in_maps = _prep_inputs(x, idx_a, idx_b, ws)
    nc = _build_bass()
    from concourse.bass_utils import run_bass_kernel_spmd
    res = run_bass_kernel_spmd(nc, in_maps, core_ids=list(range(NCORES)))
    gamma = in_maps[0]["coef"][0:64, 4 * _NMIX]
    return _assemble([r["out"] for r in res.results], gamma)


def kernel_emulate(x, idx_a, idx_b, w0, w1, w2, w3, w4, w5, w6):
    """Pure-numpy emulation of the exact device schedule (debug aid)."""
    ws = [np.asarray(w, dtype=np.float32) for w in
          (w0, w1, w2, w3, w4, w5, w6)]
    in_maps = _prep_inputs(np.asarray(x, np.float32),
                           np.asarray(idx_a, np.int32),
                           np.asarray(idx_b, np.int32), ws)
    outs = [_emulate_core(m["xs"].reshape(NROW, ROWE), m["offs"], m["coef"])
            for m in in_maps]
    return _assemble(outs, in_maps[0]["coef"][0:64, 4 * _NMIX])


# revision 26
# speedup vs baseline: 1.0131x; 1.0015x over previous
"""Trainium2 Bass kernel for nn_LogicConv3d (differentiable-logic conv tree).

Problem (hardcoded): x [16,64,32,32] f32; idx_a/idx_b [64,900,64,3] i32;
w0..w6 [s,64,16] f32 (s = 64,32,16,8,4,2,1). Output [16,64,900,1] f32.

Math: per (kernel k, window p): gather 64 (a,b) leaf pairs from x, blend each
pair with soft-gate coefficients (softmax(w) @ GATE_M), then 6 more pairwise
tree levels.  mix(a,b) = c0 + c1*a + c2*b + c3*a*b.

v3 design (fp16 end-to-end, DVE 2x/4x perf modes):
 - F-sharding: core i handles batches (2i, 2i+1); pure SPMD across 8 cores.
 - Host builds a 576-row fp16 crop table XS[(c,ha,wa), 1920]: row = the
   30x30x2 (h,w,b-interleaved) crop of channel c at shift (ha,wa), compact in
   the first 1800 elements.  Leaf gathers are indirect DMAs with per-lane
   element offsets row*1920, fetching 1800 contiguous fp16 -> operands are
   step-1 fp16, which unlocks DVE packed modes.
 - scalar_tensor_tensor has NO DVE perf modes (1x only) so the mix avoids it:
     p = a*alpha + beta   (tensor_scalar 4x on DVE, or ACT activation)
     q = b * p            (tensor_tensor mult, 2x on DVE; some on Pool)
     u = a*gamma          (tensor_scalar / ACT)
     out = q + u          (tensor_tensor add, 2x on DVE)
   alpha = c3, beta = c2 - c3*Ta, gamma = c1 - c3*Tb where Ta/Tb are the
   children's additive-bias chain (bias folding; all multiplicative, safe).
   The per-node bias T = c2*Tb + c1*Ta - c3*Ta*Tb - c0 propagates on host in
   f64; the root bias is subtracted on host.
"""
import numpy as np

B, C, H, W = 16, 64, 32, 32
K = 64
RF = 3
DEPTH = 6
S = 64
PW = 30            # windows per axis
P = PW * PW        # 900
NCORES = 8
B2 = 2             # batches per core
F = P * B2         # free size (windows x batches) = 1800
NROW = C * RF * RF          # 576 crop-table rows
ROWE = 1920                 # crop-table row stride (elements)

# static engine assignment (tuned against the HW trace; see LP in notes):
#  - all p-ops on ACT
#  - q-TT: Pool for i%16 in [0,7), else DVE
#  - u+add: fused stt on DVE, except i%4==0 mixes use CCE-DMA add with a
#    separate u tile (u on ACT, or DVE-ts when i%16==0)
def _q_on_pool(i):
    return i % 9 < 2          # ~14 q-TT ops on Pool


def _use_cce(i, level):
    return i % 8 < 3 and level < 6   # 24 mixes: add via CCE-DMA


def _u_on_act(i):
    return True               # u for CCE mixes always on ACT


def _p_on_act(i):
    return i % 8 != 3         # 56 p-ops on ACT, 8 on DVE-ts


NCOL_GATHER = 1      # L0 ops batched per indirect gather DMA (1 = per-side)

GATE_M = np.array([
    [0, 0, 0, 0], [0, 0, 0, 1], [0, 1, 0, -1], [0, 1, 0, 0],
    [0, 0, 1, -1], [0, 0, 1, 0], [0, 1, 1, -2], [0, 1, 1, -1],
    [1, -1, -1, 1], [1, -1, -1, 2], [1, 0, -1, 0], [1, 0, -1, 1],
    [1, -1, 0, 0], [1, -1, 0, 1], [1, 0, 0, -1], [1, 0, 0, 0],
], dtype=np.float32)  # [16 gates, 4] -> c0,c1,c2,c3 = GATE_M.T @ softmax(w)


# ---------------------------------------------------------------------------
# static schedule: the merge-tree op list (DFS order keeps live tiles small)
# ---------------------------------------------------------------------------
def _build_schedule():
    """Each mix op: dict(level, key, lanes, base, node[lanes], kern[lanes]).
    L0 ops gather their own leaves; level l>=1 ops read T_{l-1}[2k],[2k+1]."""
    ops = []

    def emit(l, key):
        if l == 0:
            lanes = np.arange(128)
            ops.append(dict(level=0, key=key, lanes=128, base=0,
                            node=key + 32 * (lanes >> 6), kern=lanes & 63))
            return
        emit(l - 1, 2 * key)
        emit(l - 1, 2 * key + 1)
        lanes = np.arange(128)
        nbits_out = 6 - l
        ops.append(dict(level=l, key=key, lanes=128, base=0,
                        node=((lanes >> 6) << (nbits_out - 1)) + key,
                        kern=lanes & 63))

    emit(4, 0)
    emit(4, 1)
    # L5: one full op; node i5 = lane>>6 (a DMA then realigns the top half
    # to a base-0 tile for L6's equal-base inputs)
    lanes = np.arange(128)
    ops.append(dict(level=5, key=0, lanes=128, base=0,
                    node=lanes >> 6, kern=lanes & 63))
    lanes = np.arange(64)
    ops.append(dict(level=6, key=0, lanes=64, base=0,
                    node=np.zeros(64, np.int64), kern=lanes))
    return ops


_SCHED = _build_schedule()
_NMIX = len(_SCHED)          # 64
_NCOLS = 4 * _NMIX + 4       # + final root-bias column block


def _softmax_f32(w):
    w = w.astype(np.float64)
    m = w.max(-1, keepdims=True)
    e = np.exp(w - m)
    return e / e.sum(-1, keepdims=True)


def _coef_tables(ws):
    """ws = [w0..w6]. Returns coef matrix [128, _NCOLS] f32 with per-op scalar
    columns (alpha, beta, gamma, 0) and the final root-bias column
    (value to ADD on host: -T_root)."""
    cs = []
    for wl in ws:
        p = _softmax_f32(wl)                      # [s, K, 16] f64
        cs.append(np.einsum('skg,gj->skj', p, GATE_M.astype(np.float64)))
    # bias chain: T[l][node, kern] = delivered - true value at level-l output
    T = [None] * 7
    for l in range(7):
        c0, c1, c2, c3 = (cs[l][:, :, j] for j in range(4))
        if l == 0:
            Ta = np.zeros_like(c0)
            Tb = np.zeros_like(c0)
        else:
            Ta = T[l - 1][0::2]
            Tb = T[l - 1][1::2]
        T[l] = c2 * Tb + c1 * Ta - c3 * Ta * Tb - c0
    coef = np.zeros((128, _NCOLS), dtype=np.float64)
    for i, op in enumerate(_SCHED):
        l, node, kern = op['level'], op['node'], op['kern']
        rows = op['base'] + np.arange(op['lanes'])
        c = cs[l][node, kern]                     # [lanes, 4] = c0,c1,c2,c3
        if l == 0:
            Ta = np.zeros(op['lanes'])
            Tb = np.zeros(op['lanes'])
        else:
            Ta = T[l - 1][2 * node, kern]
            Tb = T[l - 1][2 * node + 1, kern]
        coef[rows, 4 * i + 0] = c[:, 3]                      # alpha = c3
        coef[rows, 4 * i + 1] = c[:, 2] - c[:, 3] * Ta       # beta
        coef[rows, 4 * i + 2] = c[:, 1] - c[:, 3] * Tb       # gamma
    coef[0:64, 4 * _NMIX] = -T[6][0, :]                      # final add
    return coef.astype(np.float32)


def _offset_tables(idx_a, idx_b):
    """Indirect-gather element-offset tables [128, 64] i32: col = 2*t + side.
    Offset = (c*9 + ha*3 + wa) * ROWE into the fp16 crop table."""
    offs = np.zeros((128, 64), dtype=np.int64)
    for op in _SCHED:
        if op['level'] != 0:
            continue
        t = op['key']
        for side, idx in ((0, idx_a), (1, idx_b)):
            ha = idx[op['kern'], 0, op['node'], 0].astype(np.int64)
            wa = idx[op['kern'], 0, op['node'], 1].astype(np.int64)
            ca = idx[op['kern'], 0, op['node'], 2].astype(np.int64)
            offs[:, 2 * t + side] = (ca * 9 + ha * 3 + wa) * ROWE
    return offs.astype(np.int32)


def _crop_table(xs):
    """xs: [C, H, W, B2] f32 b-interleaved slice -> XS [576, 1920] fp16."""
    XS = np.zeros((NROW, ROWE), dtype=np.float16)
    for ha in range(RF):
        for wa in range(RF):
            rows = np.arange(C) * 9 + ha * 3 + wa
            XS[rows, :F] = xs[:, ha:ha + PW, wa:wa + PW, :].reshape(
                C, F).astype(np.float16)
    return XS


# ---------------------------------------------------------------------------
# numpy emulator (mirrors the device schedule incl. fp16 rounding)
# ---------------------------------------------------------------------------
def _emulate_core(XS, offs, coef):
    """XS: [576,1920] fp16; offs: [128, 64] i32. Returns [64, F] f32."""
    f16 = np.float16
    XSf = XS.reshape(-1)
    tiles = {}
    for i, op in enumerate(_SCHED):
        l, key, n, base = op['level'], op['key'], op['lanes'], op['base']
        rws = base + np.arange(n)
        al = coef[rws, 4 * i + 0][:, None].astype(np.float32)
        be = coef[rws, 4 * i + 1][:, None].astype(np.float32)
        ga = coef[rws, 4 * i + 2][:, None].astype(np.float32)
        if l == 0:
            a = np.stack([XSf[o:o + F] for o in offs[:, 2 * key]])
            b = np.stack([XSf[o:o + F] for o in offs[:, 2 * key + 1]])
            a = a.astype(np.float32)
            b = b.astype(np.float32)
        elif l < 5:
            a = tiles[(l - 1, 2 * key)].astype(np.float32)
            b = tiles[(l - 1, 2 * key + 1)].astype(np.float32)
        elif l == 5:
            a = tiles[(4, 0)].astype(np.float32)
            b = tiles[(4, 1)].astype(np.float32)
        else:
            a = tiles['T5'][0:64].astype(np.float32)
            b = tiles['T5'][64:128].astype(np.float32)
        p = f16(a * al + be).astype(np.float32)
        q = f16(b * p).astype(np.float32)
        u = f16(a * ga).astype(np.float32)
        r = f16(q + u)
        if l == 5:
            tiles['T5'] = r
        else:
            tiles[(l, key)] = r
    return tiles[(6, 0)].astype(np.float32)


# ---------------------------------------------------------------------------
# custom DVE ops: the whole mix in 2 fused DVE instructions, with
# hand-authored 2x_1p perf-mode uop programs (the stock lower() only emits
# the 1x program; without uops_2x a custom op runs at 1 elem/cycle/lane).
#   MIX_PQ : out = (Src0*C0 + C1) * Src1     [q = (a*alpha + beta) * b]
#   AFF_ADD: out =  Src0*C0 + Src1           [r = a*gamma + q]
# 2x program structure (cribbed from the stock TENSOR_TENSOR 2X_1PORT entry):
# crossbar lane 0 feeds the ALU chain, lanes 1..6 the delay regs d0..d5;
# the LO chain computes on slices 0..k-1 while HI operands ride the delay
# regs; the HI chain computes on slices k..2k-1 while the LO result is
# captured into d5; WR0_LO <- DELAY_5, WR0_HI <- ALU_OUT.
# ---------------------------------------------------------------------------
_CUSTOM_REG = {}


def _register_custom_ops():
    if _CUSTOM_REG:
        return _CUSTOM_REG
    import concourse.dve_ops as dve_ops
    from concourse.dve_ops import DveOp
    from concourse.dve_spec import Spec, Src0, Src1, C0, C1, lower
    from concourse.dve_uop import (
        DveOpSpec, UopConfig, UopDpConfig, InpSel, OutSel, OutPath,
        AluInp, DelayInp, AluOp, Trigger)

    KEEP = DelayInp.PREV_DELAY
    CAPT = DelayInp.PREV_ALU_OUT
    PA = AluInp.PREV_ALU_OUT
    PD = [AluInp.PREV_DELAY_0, AluInp.PREV_DELAY_1, AluInp.PREV_DELAY_2,
          AluInp.PREV_DELAY_3, AluInp.PREV_DELAY_4, AluInp.PREV_DELAY_5]

    def dp(op=AluOp.BYPASS, s0=PA, s1=PA, keep=(), capt5=False):
        delay = [KEEP if k in keep else DelayInp.PREV_ALU_OUT for k in range(7)]
        den = [1 if k in keep else 0 for k in range(7)]
        if capt5:
            delay[5] = CAPT
            den[5] = 1
        return UopDpConfig(op=op, alu_src0=s0, alu_src1=s1, delay=delay,
                           alu_out_enable=1, delay_enable=den)

    def mk2x(lanes, stages, n_in):
        """lanes: 8 InpSel; stages: list of per-slice dp configs."""
        inp_en = [1 if lanes[k] != InpSel.ZERO or k == 0 else 0
                  for k in range(8)]
        return UopConfig(
            inp=lanes, inp_enable=inp_en,
            out={OutPath.WR0_LO: OutSel.DELAY_5,
                 OutPath.WR0_HI: OutSel.ALU_OUT,
                 OutPath.WR1_LO: OutSel.ALU_OUT,
                 OutPath.WR1_HI: OutSel.ALU_OUT},
            out_enable={OutPath.WR0_LO: 1, OutPath.WR0_HI: 1,
                        OutPath.WR1_LO: 0, OutPath.WR1_HI: 0},
            require_inp0=1, require_inp1=1,
            trigger=(Trigger.SRC_TENSOR_DONE, Trigger.NONE, Trigger.NONE),
            datapath_config=stages)

    M, A, BP = AluOp.MULTIPLY, AluOp.ADD, AluOp.BYPASS
    Z = InpSel.ZERO

    # ---- MIX_PQ: lanes: alu=SRC_0, d0=C0, d1=C1, d2=SRC_1, d3=SRC_0_HI,
    #      d4=SRC_1_HI
    mixpq_2x = mk2x(
        [InpSel.SRC_0, InpSel.CONST_0, InpSel.CONST_1, InpSel.SRC_1,
         InpSel.SRC_0_HI, InpSel.SRC_1_HI, Z, Z],
        [
            dp(M, PA, PD[0], keep=(0, 1, 2, 3, 4)),       # m_lo = a*C0
            dp(A, PA, PD[1], keep=(0, 1, 2, 3, 4)),       # a_lo = m_lo+C1
            dp(M, PA, PD[2], keep=(0, 1, 3, 4)),          # q_lo = a_lo*b
            dp(M, PD[3], PD[0], keep=(0, 1, 4), capt5=True),  # m_hi; d5<-q_lo
            dp(A, PA, PD[1], keep=(4, 5)),                # a_hi = m_hi+C1
            dp(M, PA, PD[4], keep=(5,)),                  # q_hi = a_hi*b_hi
            dp(BP, PA, PA, keep=(5,)),
            dp(BP, PA, PA, keep=(5,)),
        ], 6)

    # ---- AFF_ADD: lanes: alu=SRC_0, d0=C0, d1=SRC_1, d2=SRC_0_HI,
    #      d3=SRC_1_HI
    affadd_2x = mk2x(
        [InpSel.SRC_0, InpSel.CONST_0, InpSel.SRC_1, InpSel.SRC_0_HI,
         InpSel.SRC_1_HI, Z, Z, Z],
        [
            dp(M, PA, PD[0], keep=(0, 1, 2, 3)),          # m_lo = a*C0
            dp(A, PA, PD[1], keep=(0, 2, 3)),             # r_lo = m_lo+q
            dp(M, PD[2], PD[0], keep=(3,), capt5=True),   # m_hi; d5<-r_lo
            dp(A, PA, PD[3], keep=(5,)),                  # r_hi = m_hi+q_hi
            dp(BP, PA, PA, keep=(5,)),
            dp(BP, PA, PA, keep=(5,)),
            dp(BP, PA, PA, keep=(5,)),
            dp(BP, PA, PA, keep=(5,)),
        ], 5)

    defs = [
        ("ANT_LC_MIX_PQ",
         Spec(body=(Src0 * C0 + C1) * Src1,
              reference=lambda in0, in1, s0, s1, imm2:
              ((in0.astype(np.float32) * s0 + s1) * in1)),
         mixpq_2x),
        ("ANT_LC_AFF_ADD",
         Spec(body=Src0 * C0 + Src1,
              reference=lambda in0, in1, s0, s1, imm2:
              (in0.astype(np.float32) * s0 + in1)),
         affadd_2x),
    ]
    ver = "v3"
    for name, spec, u2x in defs:
        row = 1 + len(dve_ops.OPS)
        assert row < 0x20
        uops_1x = lower(spec, ver=ver)
        compiled = DveOpSpec(name=name, opcode=row, uops=uops_1x,
                             uops_2x=[u2x], rd1_en=True, perf_max=1)
        compiled.validate(ver)
        op = DveOp(name, spec, subdim=False,
                   uops_sha={ver: compiled.sha(ver)})
        dve_ops.OPS.append(op)
        dve_ops.CUSTOM_DVE_SPECS[name] = spec
        dve_ops._SUB_OPCODE_FOR_NAME[name] = row
        dve_ops._COMPILE_CACHE[(name, ver)] = compiled
        _CUSTOM_REG[name] = (op, row)
    return _CUSTOM_REG


def _emit_custom(nc, name, out, in0, in1, s0, s1=0.0, perf_max=1):
    """Emit one custom-DVE instruction (like bass Vector._custom_dve but
    with perf_max set so the engine may use the 2x_1p program)."""
    import concourse.bass_isa as bass_isa
    import concourse.mybir as mybir
    from concourse.dve_ops import get_dve_sub_opcode
    vec = nc.vector
    if name not in nc.m.ant_custom_dve_ops:
        nc.m.ant_custom_dve_ops = sorted({*nc.m.ant_custom_dve_ops, name})
    shape = bass_isa.CustomDveShape.TTSS
    isa_opcode = nc.isa.Opcode[
        f"NEURON_ISA_TPB_OPCODE_CUSTOM_DVE_ANT_{shape.slot()}"].value

    def lo_scalar(v):
        if isinstance(v, (int, float)):
            return mybir.ImmediateValue(dtype=mybir.dt.float32, value=float(v))
        return vec.lower_ap(v, for_isa=True)

    ins = [vec.lower_ap(in0, for_isa=True, opt=True),
           vec.lower_ap(in1, for_isa=True, opt=True),
           lo_scalar(s0), lo_scalar(s1)]
    outs = [vec.lower_ap(out, for_isa=True, opt=True)]
    return vec.add_instruction(
        bass_isa.InstCustomDveAnt(
            name=nc.get_next_instruction_name(),
            op_name=name, rd1_en=True, subdim=0, imm2=0.0, shape=shape,
            row=get_dve_sub_opcode(name), isa_opcode=isa_opcode,
            perf_max=perf_max, ins=ins, outs=outs))


# ---------------------------------------------------------------------------
# Bass program (built once, cached)
# ---------------------------------------------------------------------------
_BASS_CACHE = {}


def _build_bass():
    if 'nc' in _BASS_CACHE:
        return _BASS_CACHE['nc']
    import concourse.bass as bass
    import concourse.mybir as mybir
    import concourse.tile as tile
    import concourse.bacc as bacc

    _register_custom_ops()
    f32 = mybir.dt.float32
    f16 = mybir.dt.float16
    nc = bacc.Bacc("TRN2", target_bir_lowering=False, debug=False,
                   num_devices=NCORES)
    nxs = NROW * ROWE
    xs_d = nc.dram_tensor("xs", [nxs, 1], f16, kind="ExternalInput").ap()
    offs_d = nc.dram_tensor("offs", [128, 64], mybir.dt.int32,
                            kind="ExternalInput").ap()
    coef_d = nc.dram_tensor("coef", [128, _NCOLS], f32,
                            kind="ExternalInput").ap()
    out_d = nc.dram_tensor("out", [64, F], f16, kind="ExternalOutput").ap()

    AL = mybir.AluOpType
    ACTF = mybir.ActivationFunctionType

    with tile.TileContext(nc) as tc:
        with (
            tc.tile_pool(name="const", bufs=1) as pc,
            tc.tile_pool(name="ab", bufs=10) as pab,
            tc.tile_pool(name="lvl", bufs=3) as plv,
            tc.tile_pool(name="t0p", bufs=3) as pt0,
            tc.tile_pool(name="tmp", bufs=8) as ptmp,
            tc.tile_pool(name="fin", bufs=1) as pfin,
        ):
            offs_t = pc.tile([128, 64], mybir.dt.int32, tag="offs",
                             name="offs_t")
            nc.sync.dma_start(offs_t[:], offs_d[:])
            coef_t = pc.tile([128, _NCOLS], f32, tag="coef", name="coef_t")
            nc.sync.dma_start(coef_t[:], coef_d[:])
            warm_t = pc.tile([1, 8], f32, tag="warm", name="warm_t")
            nc.scalar.activation(warm_t[:], coef_t[0:1, 0:8],
                                 ACTF.Identity, bias=0.0, scale=1.0)

            tiles = {}
            gtiles = {}
            for i, op in enumerate(_SCHED):
                l, key, n, base = op['level'], op['key'], op['lanes'], op['base']
                sl = slice(base, base + n)
                al = coef_t[sl, 4 * i + 0:4 * i + 1]
                be = coef_t[sl, 4 * i + 1:4 * i + 2]
                ga = coef_t[sl, 4 * i + 2:4 * i + 3]
                if l == 0:
                    if NCOL_GATHER > 1:
                        gk = (key // NCOL_GATHER) * NCOL_GATHER
                        if key == gk:
                            g_t = pab.tile([128, 2 * NCOL_GATHER * F], f16,
                                           tag="AB", name="ab_t")
                            g_ap = g_t[:].rearrange(
                                "p (j e) -> p j e", j=2 * NCOL_GATHER, e=F)
                            nc.gpsimd.indirect_dma_start(
                                out=g_ap, out_offset=None, in_=xs_d[:],
                                in_offset=bass.IndirectOffsetOnAxis(
                                    ap=offs_t[:, 2 * gk:
                                              2 * (gk + NCOL_GATHER)],
                                    axis=0))
                            for kk in range(gk, gk + NCOL_GATHER):
                                gtiles[kk] = g_t
                        g_t = gtiles[key]
                        half = key - gk
                        a_ap = g_t[:, 2 * half * F:(2 * half + 1) * F]
                        b_ap = g_t[:, (2 * half + 1) * F:(2 * half + 2) * F]
                    else:
                        a_t = pab.tile([128, F], f16, tag="A", name="at")
                        b_t = pab.tile([128, F], f16, tag="B", name="bt")
                        for side, dst in ((0, a_t), (1, b_t)):
                            nc.gpsimd.indirect_dma_start(
                                out=dst[:], out_offset=None, in_=xs_d[:],
                                in_offset=bass.IndirectOffsetOnAxis(
                                    ap=offs_t[:, 2 * key + side:
                                              2 * key + side + 1], axis=0))
                        a_ap, b_ap = a_t[:, :], b_t[:, :]
                elif l < 5:
                    a_ap = tiles[(l - 1, 2 * key)][:, :]
                    b_ap = tiles[(l - 1, 2 * key + 1)][:, :]
                elif l == 5:
                    a_ap = tiles[(4, 0)][:, :]
                    b_ap = tiles[(4, 1)][:, :]
                else:
                    a_ap = tiles['T5'][0:64, :]
                    b_ap = tiles['T5b'][:, :]

                # q = (a*alpha + beta) * b   [fused custom DVE op, 2x]
                # (offload experiments regressed: ops with upstream compute
                # deps stall the in-order Pool engine and block later gather
                # descgens -- keep Pool gather-only, all mixes on DVE)
                # (h-splitting the first mix / L5-L6 tail was tried and
                # regressed: extra per-op overhead + early-pipeline stalls
                # outweighed the ~1us tail-gap gain)
                hsp = ((0, 900), (900, 900)) if l == 6 else ((0, F),)
                p_t = ptmp.tile([n, F], f16, tag="p", name="p")
                _emit_custom(nc, "ANT_LC_MIX_PQ", p_t[:, :], a_ap, b_ap,
                             al, be)
                # output tile
                if l == 5:
                    r_t = pfin.tile([128, F], f16, tag="T5", name="t5")
                    tiles['T5'] = r_t
                elif l == 6:
                    r_t = pfin.tile([64, F], f16, tag="T6", name="t6")
                else:
                    pool = pt0 if l == 0 else plv
                    r_t = pool.tile([128, F], f16, tag=f"T{l}",
                                    name=f"t{l}_{key}")
                    tiles[(l, key)] = r_t

                if l == 5:
                    t5b = pfin.tile([64, F], f16, tag="T5b", name="t5b")
                    tiles['T5b'] = t5b
                for off, ln in hsp:
                    fs = slice(off, off + ln)
                    _emit_custom(nc, "ANT_LC_AFF_ADD", r_t[:, fs],
                                 a_ap[:, fs], p_t[:, fs], ga)
                    if l == 5:
                        nc.sync.dma_start(tiles['T5b'][:, fs],
                                          r_t[64:128, fs])
                    elif l == 6:
                        nc.sync.dma_start(out_d[:, fs], r_t[:, fs])
    nc.compile()
    _BASS_CACHE['nc'] = nc
    return nc


def _prep_inputs(x, idx_a, idx_b, ws):
    coef = _coef_tables(ws)
    offs = _offset_tables(idx_a, idx_b)
    x = np.ascontiguousarray(x, dtype=np.float32)
    in_maps = []
    for core in range(NCORES):
        xs = x[B2 * core:B2 * core + B2].transpose(1, 2, 3, 0)  # [C,H,W,B2]
        in_maps.append({"xs": _crop_table(xs).reshape(-1, 1),
                        "offs": offs, "coef": coef})
    return in_maps


def _assemble(core_outs, gamma):
    """core_outs: list of [64, F=(hh,ww,b)]; gamma [64] -> [16,64,900,1]."""
    full = np.stack([np.asarray(o, dtype=np.float32) for o in core_outs])
    full = full + gamma.astype(np.float32)[None, :, None]
    full = full.reshape(NCORES, K, P, B2)           # [core, k, p, b_local]
    full = full.transpose(0, 3, 1, 2).reshape(B, K, P, 1)
    return np.ascontiguousarray(full.astype(np.float32))


def kernel(x, idx_a, idx_b, w0, w1, w2, w3, w4, w5, w6):
    ws = [np.asarray(w, dtype=np.float32) for w in
          (w0, w1, w2, w3, w4, w5, w6)]
    x = np.asarray(x, dtype=np.float32)
    idx_a = np.asarray(idx_a, dtype=np.int32)
    idx_b = np.asarray(idx_b, dtype=np.int32)
    in_maps = _prep_inputs(x, idx_a, idx_b, ws)
    nc = _build_bass()
    from concourse.bass_utils import run_bass_kernel_spmd
    res = run_bass_kernel_spmd(nc, in_maps, core_ids=list(range(NCORES)))
    gamma = in_maps[0]["coef"][0:64, 4 * _NMIX]
    return _assemble([r["out"] for r in res.results], gamma)


def kernel_emulate(x, idx_a, idx_b, w0, w1, w2, w3, w4, w5, w6):
    """Pure-numpy emulation of the exact device schedule (debug aid)."""
    ws = [np.asarray(w, dtype=np.float32) for w in
          (w0, w1, w2, w3, w4, w5, w6)]
    in_maps = _prep_inputs(np.asarray(x, np.float32),
                           np.asarray(idx_a, np.int32),
                           np.asarray(idx_b, np.int32), ws)
    outs = [_emulate_core(m["xs"].reshape(NROW, ROWE), m["offs"], m["coef"])
            for m in in_maps]
    return _assemble(outs, in_maps[0]["coef"][0:64, 4 * _NMIX])


# revision 29
# speedup vs baseline: 1.0135x; 1.0003x over previous
"""Trainium2 Bass kernel for nn_LogicConv3d (differentiable-logic conv tree).

Problem (hardcoded): x [16,64,32,32] f32; idx_a/idx_b [64,900,64,3] i32;
w0..w6 [s,64,16] f32 (s = 64,32,16,8,4,2,1). Output [16,64,900,1] f32.

Math: per (kernel k, window p): gather 64 (a,b) leaf pairs from x, blend each
pair with soft-gate coefficients (softmax(w) @ GATE_M), then 6 more pairwise
tree levels.  mix(a,b) = c0 + c1*a + c2*b + c3*a*b.

v3 design (fp16 end-to-end, DVE 2x/4x perf modes):
 - F-sharding: core i handles batches (2i, 2i+1); pure SPMD across 8 cores.
 - Host builds a 576-row fp16 crop table XS[(c,ha,wa), 1920]: row = the
   30x30x2 (h,w,b-interleaved) crop of channel c at shift (ha,wa), compact in
   the first 1800 elements.  Leaf gathers are indirect DMAs with per-lane
   element offsets row*1920, fetching 1800 contiguous fp16 -> operands are
   step-1 fp16, which unlocks DVE packed modes.
 - scalar_tensor_tensor has NO DVE perf modes (1x only) so the mix avoids it:
     p = a*alpha + beta   (tensor_scalar 4x on DVE, or ACT activation)
     q = b * p            (tensor_tensor mult, 2x on DVE; some on Pool)
     u = a*gamma          (tensor_scalar / ACT)
     out = q + u          (tensor_tensor add, 2x on DVE)
   alpha = c3, beta = c2 - c3*Ta, gamma = c1 - c3*Tb where Ta/Tb are the
   children's additive-bias chain (bias folding; all multiplicative, safe).
   The per-node bias T = c2*Tb + c1*Ta - c3*Ta*Tb - c0 propagates on host in
   f64; the root bias is subtracted on host.
"""
import numpy as np

B, C, H, W = 16, 64, 32, 32
K = 64
RF = 3
DEPTH = 6
S = 64
PW = 30            # windows per axis
P = PW * PW        # 900
NCORES = 8
B2 = 2             # batches per core
F = P * B2         # free size (windows x batches) = 1800
NROW = C * RF * RF          # 576 crop-table rows
ROWE = 1920                 # crop-table row stride (elements)

# static engine assignment (tuned against the HW trace; see LP in notes):
#  - all p-ops on ACT
#  - q-TT: Pool for i%16 in [0,7), else DVE
#  - u+add: fused stt on DVE, except i%4==0 mixes use CCE-DMA add with a
#    separate u tile (u on ACT, or DVE-ts when i%16==0)
def _q_on_pool(i):
    return i % 9 < 2          # ~14 q-TT ops on Pool


def _use_cce(i, level):
    return i % 8 < 3 and level < 6   # 24 mixes: add via CCE-DMA


def _u_on_act(i):
    return True               # u for CCE mixes always on ACT


def _p_on_act(i):
    return i % 8 != 3         # 56 p-ops on ACT, 8 on DVE-ts


NCOL_GATHER = 1      # L0 ops batched per indirect gather DMA (1 = per-side)

GATE_M = np.array([
    [0, 0, 0, 0], [0, 0, 0, 1], [0, 1, 0, -1], [0, 1, 0, 0],
    [0, 0, 1, -1], [0, 0, 1, 0], [0, 1, 1, -2], [0, 1, 1, -1],
    [1, -1, -1, 1], [1, -1, -1, 2], [1, 0, -1, 0], [1, 0, -1, 1],
    [1, -1, 0, 0], [1, -1, 0, 1], [1, 0, 0, -1], [1, 0, 0, 0],
], dtype=np.float32)  # [16 gates, 4] -> c0,c1,c2,c3 = GATE_M.T @ softmax(w)


# ---------------------------------------------------------------------------
# static schedule: the merge-tree op list (DFS order keeps live tiles small)
# ---------------------------------------------------------------------------
def _build_schedule():
    """Each mix op: dict(level, key, lanes, base, node[lanes], kern[lanes]).
    L0 ops gather their own leaves; level l>=1 ops read T_{l-1}[2k],[2k+1]."""
    ops = []

    def emit(l, key):
        if l == 0:
            lanes = np.arange(128)
            ops.append(dict(level=0, key=key, lanes=128, base=0,
                            node=key + 32 * (lanes >> 6), kern=lanes & 63))
            return
        emit(l - 1, 2 * key)
        emit(l - 1, 2 * key + 1)
        lanes = np.arange(128)
        nbits_out = 6 - l
        ops.append(dict(level=l, key=key, lanes=128, base=0,
                        node=((lanes >> 6) << (nbits_out - 1)) + key,
                        kern=lanes & 63))

    emit(4, 0)
    emit(4, 1)
    # L5: one full op; node i5 = lane>>6 (a DMA then realigns the top half
    # to a base-0 tile for L6's equal-base inputs)
    lanes = np.arange(128)
    ops.append(dict(level=5, key=0, lanes=128, base=0,
                    node=lanes >> 6, kern=lanes & 63))
    lanes = np.arange(64)
    ops.append(dict(level=6, key=0, lanes=64, base=0,
                    node=np.zeros(64, np.int64), kern=lanes))
    return ops


_SCHED = _build_schedule()
_NMIX = len(_SCHED)          # 64
_NCOLS = 4 * _NMIX + 4       # + final root-bias column block


def _softmax_f32(w):
    w = w.astype(np.float64)
    m = w.max(-1, keepdims=True)
    e = np.exp(w - m)
    return e / e.sum(-1, keepdims=True)


def _coef_tables(ws):
    """ws = [w0..w6]. Returns coef matrix [128, _NCOLS] f32 with per-op scalar
    columns (alpha, beta, gamma, 0) and the final root-bias column
    (value to ADD on host: -T_root)."""
    cs = []
    for wl in ws:
        p = _softmax_f32(wl)                      # [s, K, 16] f64
        cs.append(np.einsum('skg,gj->skj', p, GATE_M.astype(np.float64)))
    # bias chain: T[l][node, kern] = delivered - true value at level-l output
    T = [None] * 7
    for l in range(7):
        c0, c1, c2, c3 = (cs[l][:, :, j] for j in range(4))
        if l == 0:
            Ta = np.zeros_like(c0)
            Tb = np.zeros_like(c0)
        else:
            Ta = T[l - 1][0::2]
            Tb = T[l - 1][1::2]
        T[l] = c2 * Tb + c1 * Ta - c3 * Ta * Tb - c0
    coef = np.zeros((128, _NCOLS), dtype=np.float64)
    for i, op in enumerate(_SCHED):
        l, node, kern = op['level'], op['node'], op['kern']
        rows = op['base'] + np.arange(op['lanes'])
        c = cs[l][node, kern]                     # [lanes, 4] = c0,c1,c2,c3
        if l == 0:
            Ta = np.zeros(op['lanes'])
            Tb = np.zeros(op['lanes'])
        else:
            Ta = T[l - 1][2 * node, kern]
            Tb = T[l - 1][2 * node + 1, kern]
        coef[rows, 4 * i + 0] = c[:, 3]                      # alpha = c3
        coef[rows, 4 * i + 1] = c[:, 2] - c[:, 3] * Ta       # beta
        coef[rows, 4 * i + 2] = c[:, 1] - c[:, 3] * Tb       # gamma
    coef[0:64, 4 * _NMIX] = -T[6][0, :]                      # final add
    return coef.astype(np.float32)


def _offset_tables(idx_a, idx_b):
    """Indirect-gather element-offset tables [128, 64] i32: col = 2*t + side.
    Offset = (c*9 + ha*3 + wa) * ROWE into the fp16 crop table."""
    offs = np.zeros((128, 64), dtype=np.int64)
    for op in _SCHED:
        if op['level'] != 0:
            continue
        t = op['key']
        for side, idx in ((0, idx_a), (1, idx_b)):
            ha = idx[op['kern'], 0, op['node'], 0].astype(np.int64)
            wa = idx[op['kern'], 0, op['node'], 1].astype(np.int64)
            ca = idx[op['kern'], 0, op['node'], 2].astype(np.int64)
            offs[:, 2 * t + side] = (ca * 9 + ha * 3 + wa) * ROWE
    return offs.astype(np.int32)


def _crop_table(xs):
    """xs: [C, H, W, B2] f32 b-interleaved slice -> XS [576, 1920] fp16."""
    XS = np.zeros((NROW, ROWE), dtype=np.float16)
    for ha in range(RF):
        for wa in range(RF):
            rows = np.arange(C) * 9 + ha * 3 + wa
            XS[rows, :F] = xs[:, ha:ha + PW, wa:wa + PW, :].reshape(
                C, F).astype(np.float16)
    return XS


# ---------------------------------------------------------------------------
# numpy emulator (mirrors the device schedule incl. fp16 rounding)
# ---------------------------------------------------------------------------
def _emulate_core(XS, offs, coef):
    """XS: [576,1920] fp16; offs: [128, 64] i32. Returns [64, F] f32."""
    f16 = np.float16
    XSf = XS.reshape(-1)
    tiles = {}
    for i, op in enumerate(_SCHED):
        l, key, n, base = op['level'], op['key'], op['lanes'], op['base']
        rws = base + np.arange(n)
        al = coef[rws, 4 * i + 0][:, None].astype(np.float32)
        be = coef[rws, 4 * i + 1][:, None].astype(np.float32)
        ga = coef[rws, 4 * i + 2][:, None].astype(np.float32)
        if l == 0:
            a = np.stack([XSf[o:o + F] for o in offs[:, 2 * key]])
            b = np.stack([XSf[o:o + F] for o in offs[:, 2 * key + 1]])
            a = a.astype(np.float32)
            b = b.astype(np.float32)
        elif l < 5:
            a = tiles[(l - 1, 2 * key)].astype(np.float32)
            b = tiles[(l - 1, 2 * key + 1)].astype(np.float32)
        elif l == 5:
            a = tiles[(4, 0)].astype(np.float32)
            b = tiles[(4, 1)].astype(np.float32)
        else:
            a = tiles['T5'][0:64].astype(np.float32)
            b = tiles['T5'][64:128].astype(np.float32)
        p = f16(a * al + be).astype(np.float32)
        q = f16(b * p).astype(np.float32)
        u = f16(a * ga).astype(np.float32)
        r = f16(q + u)
        if l == 5:
            tiles['T5'] = r
        else:
            tiles[(l, key)] = r
    return tiles[(6, 0)].astype(np.float32)


# ---------------------------------------------------------------------------
# custom DVE ops: the whole mix in 2 fused DVE instructions, with
# hand-authored 2x_1p perf-mode uop programs (the stock lower() only emits
# the 1x program; without uops_2x a custom op runs at 1 elem/cycle/lane).
#   MIX_PQ : out = (Src0*C0 + C1) * Src1     [q = (a*alpha + beta) * b]
#   AFF_ADD: out =  Src0*C0 + Src1           [r = a*gamma + q]
# 2x program structure (cribbed from the stock TENSOR_TENSOR 2X_1PORT entry):
# crossbar lane 0 feeds the ALU chain, lanes 1..6 the delay regs d0..d5;
# the LO chain computes on slices 0..k-1 while HI operands ride the delay
# regs; the HI chain computes on slices k..2k-1 while the LO result is
# captured into d5; WR0_LO <- DELAY_5, WR0_HI <- ALU_OUT.
# ---------------------------------------------------------------------------
_CUSTOM_REG = {}


def _register_custom_ops():
    if _CUSTOM_REG:
        return _CUSTOM_REG
    import concourse.dve_ops as dve_ops
    from concourse.dve_ops import DveOp
    from concourse.dve_spec import Spec, Src0, Src1, C0, C1, lower
    from concourse.dve_uop import (
        DveOpSpec, UopConfig, UopDpConfig, InpSel, OutSel, OutPath,
        AluInp, DelayInp, AluOp, Trigger)

    KEEP = DelayInp.PREV_DELAY
    CAPT = DelayInp.PREV_ALU_OUT
    PA = AluInp.PREV_ALU_OUT
    PD = [AluInp.PREV_DELAY_0, AluInp.PREV_DELAY_1, AluInp.PREV_DELAY_2,
          AluInp.PREV_DELAY_3, AluInp.PREV_DELAY_4, AluInp.PREV_DELAY_5]

    def dp(op=AluOp.BYPASS, s0=PA, s1=PA, keep=(), capt5=False):
        delay = [KEEP if k in keep else DelayInp.PREV_ALU_OUT for k in range(7)]
        den = [1 if k in keep else 0 for k in range(7)]
        if capt5:
            delay[5] = CAPT
            den[5] = 1
        return UopDpConfig(op=op, alu_src0=s0, alu_src1=s1, delay=delay,
                           alu_out_enable=1, delay_enable=den)

    def mk2x(lanes, stages, n_in):
        """lanes: 8 InpSel; stages: list of per-slice dp configs."""
        inp_en = [1 if lanes[k] != InpSel.ZERO or k == 0 else 0
                  for k in range(8)]
        return UopConfig(
            inp=lanes, inp_enable=inp_en,
            out={OutPath.WR0_LO: OutSel.DELAY_5,
                 OutPath.WR0_HI: OutSel.ALU_OUT,
                 OutPath.WR1_LO: OutSel.ALU_OUT,
                 OutPath.WR1_HI: OutSel.ALU_OUT},
            out_enable={OutPath.WR0_LO: 1, OutPath.WR0_HI: 1,
                        OutPath.WR1_LO: 0, OutPath.WR1_HI: 0},
            require_inp0=1, require_inp1=1,
            trigger=(Trigger.SRC_TENSOR_DONE, Trigger.NONE, Trigger.NONE),
            datapath_config=stages)

    M, A, BP = AluOp.MULTIPLY, AluOp.ADD, AluOp.BYPASS
    Z = InpSel.ZERO

    # ---- MIX_PQ: lanes: alu=SRC_0, d0=C0, d1=C1, d2=SRC_1, d3=SRC_0_HI,
    #      d4=SRC_1_HI
    mixpq_2x = mk2x(
        [InpSel.SRC_0, InpSel.CONST_0, InpSel.CONST_1, InpSel.SRC_1,
         InpSel.SRC_0_HI, InpSel.SRC_1_HI, Z, Z],
        [
            dp(M, PA, PD[0], keep=(0, 1, 2, 3, 4)),       # m_lo = a*C0
            dp(A, PA, PD[1], keep=(0, 1, 2, 3, 4)),       # a_lo = m_lo+C1
            dp(M, PA, PD[2], keep=(0, 1, 3, 4)),          # q_lo = a_lo*b
            dp(M, PD[3], PD[0], keep=(0, 1, 4), capt5=True),  # m_hi; d5<-q_lo
            dp(A, PA, PD[1], keep=(4, 5)),                # a_hi = m_hi+C1
            dp(M, PA, PD[4], keep=(5,)),                  # q_hi = a_hi*b_hi
            dp(BP, PA, PA, keep=(5,)),
            dp(BP, PA, PA, keep=(5,)),
        ], 6)

    # ---- AFF_ADD: lanes: alu=SRC_0, d0=C0, d1=SRC_1, d2=SRC_0_HI,
    #      d3=SRC_1_HI
    affadd_2x = mk2x(
        [InpSel.SRC_0, InpSel.CONST_0, InpSel.SRC_1, InpSel.SRC_0_HI,
         InpSel.SRC_1_HI, Z, Z, Z],
        [
            dp(M, PA, PD[0], keep=(0, 1, 2, 3)),          # m_lo = a*C0
            dp(A, PA, PD[1], keep=(0, 2, 3)),             # r_lo = m_lo+q
            dp(M, PD[2], PD[0], keep=(3,), capt5=True),   # m_hi; d5<-r_lo
            dp(A, PA, PD[3], keep=(5,)),                  # r_hi = m_hi+q_hi
            dp(BP, PA, PA, keep=(5,)),
            dp(BP, PA, PA, keep=(5,)),
            dp(BP, PA, PA, keep=(5,)),
            dp(BP, PA, PA, keep=(5,)),
        ], 5)

    defs = [
        ("ANT_LC_MIX_PQ",
         Spec(body=(Src0 * C0 + C1) * Src1,
              reference=lambda in0, in1, s0, s1, imm2:
              ((in0.astype(np.float32) * s0 + s1) * in1)),
         mixpq_2x),
        ("ANT_LC_AFF_ADD",
         Spec(body=Src0 * C0 + Src1,
              reference=lambda in0, in1, s0, s1, imm2:
              (in0.astype(np.float32) * s0 + in1)),
         affadd_2x),
    ]
    ver = "v3"
    for name, spec, u2x in defs:
        row = 1 + len(dve_ops.OPS)
        assert row < 0x20
        uops_1x = lower(spec, ver=ver)
        compiled = DveOpSpec(name=name, opcode=row, uops=uops_1x,
                             uops_2x=[u2x], rd1_en=True, perf_max=1)
        compiled.validate(ver)
        op = DveOp(name, spec, subdim=False,
                   uops_sha={ver: compiled.sha(ver)})
        dve_ops.OPS.append(op)
        dve_ops.CUSTOM_DVE_SPECS[name] = spec
        dve_ops._SUB_OPCODE_FOR_NAME[name] = row
        dve_ops._COMPILE_CACHE[(name, ver)] = compiled
        _CUSTOM_REG[name] = (op, row)
    return _CUSTOM_REG


def _emit_custom(nc, name, out, in0, in1, s0, s1=0.0, perf_max=1):
    """Emit one custom-DVE instruction (like bass Vector._custom_dve but
    with perf_max set so the engine may use the 2x_1p program)."""
    import concourse.bass_isa as bass_isa
    import concourse.mybir as mybir
    from concourse.dve_ops import get_dve_sub_opcode
    vec = nc.vector
    if name not in nc.m.ant_custom_dve_ops:
        nc.m.ant_custom_dve_ops = sorted({*nc.m.ant_custom_dve_ops, name})
    shape = bass_isa.CustomDveShape.TTSS
    isa_opcode = nc.isa.Opcode[
        f"NEURON_ISA_TPB_OPCODE_CUSTOM_DVE_ANT_{shape.slot()}"].value

    def lo_scalar(v):
        if isinstance(v, (int, float)):
            return mybir.ImmediateValue(dtype=mybir.dt.float32, value=float(v))
        return vec.lower_ap(v, for_isa=True)

    ins = [vec.lower_ap(in0, for_isa=True, opt=True),
           vec.lower_ap(in1, for_isa=True, opt=True),
           lo_scalar(s0), lo_scalar(s1)]
    outs = [vec.lower_ap(out, for_isa=True, opt=True)]
    return vec.add_instruction(
        bass_isa.InstCustomDveAnt(
            name=nc.get_next_instruction_name(),
            op_name=name, rd1_en=True, subdim=0, imm2=0.0, shape=shape,
            row=get_dve_sub_opcode(name), isa_opcode=isa_opcode,
            perf_max=perf_max, ins=ins, outs=outs))


# ---------------------------------------------------------------------------
# Bass program (built once, cached)
# ---------------------------------------------------------------------------
_BASS_CACHE = {}


def _build_bass():
    if 'nc' in _BASS_CACHE:
        return _BASS_CACHE['nc']
    import concourse.bass as bass
    import concourse.mybir as mybir
    import concourse.tile as tile
    import concourse.bacc as bacc

    _register_custom_ops()
    f32 = mybir.dt.float32
    f16 = mybir.dt.float16
    nc = bacc.Bacc("TRN2", target_bir_lowering=False, debug=False,
                   num_devices=NCORES)
    nxs = NROW * ROWE
    xs_d = nc.dram_tensor("xs", [nxs, 1], f16, kind="ExternalInput").ap()
    offs_d = nc.dram_tensor("offs", [128, 64], mybir.dt.int32,
                            kind="ExternalInput").ap()
    coef_d = nc.dram_tensor("coef", [128, _NCOLS], f32,
                            kind="ExternalInput").ap()
    out_d = nc.dram_tensor("out", [64, F], f16, kind="ExternalOutput").ap()

    AL = mybir.AluOpType
    ACTF = mybir.ActivationFunctionType

    with tile.TileContext(nc) as tc:
        with (
            tc.tile_pool(name="const", bufs=1) as pc,
            tc.tile_pool(name="ab", bufs=12) as pab,
            tc.tile_pool(name="lvl", bufs=3) as plv,
            tc.tile_pool(name="t0p", bufs=3) as pt0,
            tc.tile_pool(name="tmp", bufs=8) as ptmp,
            tc.tile_pool(name="fin", bufs=1) as pfin,
        ):
            offs_t = pc.tile([128, 64], mybir.dt.int32, tag="offs",
                             name="offs_t")
            nc.sync.dma_start(offs_t[:], offs_d[:])
            coef_t = pc.tile([128, _NCOLS], f32, tag="coef", name="coef_t")
            nc.sync.dma_start(coef_t[:], coef_d[:])
            warm_t = pc.tile([1, 8], f32, tag="warm", name="warm_t")
            nc.scalar.activation(warm_t[:], coef_t[0:1, 0:8],
                                 ACTF.Identity, bias=0.0, scale=1.0)

            tiles = {}
            gtiles = {}
            for i, op in enumerate(_SCHED):
                l, key, n, base = op['level'], op['key'], op['lanes'], op['base']
                sl = slice(base, base + n)
                al = coef_t[sl, 4 * i + 0:4 * i + 1]
                be = coef_t[sl, 4 * i + 1:4 * i + 2]
                ga = coef_t[sl, 4 * i + 2:4 * i + 3]
                if l == 0:
                    if NCOL_GATHER > 1:
                        gk = (key // NCOL_GATHER) * NCOL_GATHER
                        if key == gk:
                            g_t = pab.tile([128, 2 * NCOL_GATHER * F], f16,
                                           tag="AB", name="ab_t")
                            g_ap = g_t[:].rearrange(
                                "p (j e) -> p j e", j=2 * NCOL_GATHER, e=F)
                            nc.gpsimd.indirect_dma_start(
                                out=g_ap, out_offset=None, in_=xs_d[:],
                                in_offset=bass.IndirectOffsetOnAxis(
                                    ap=offs_t[:, 2 * gk:
                                              2 * (gk + NCOL_GATHER)],
                                    axis=0))
                            for kk in range(gk, gk + NCOL_GATHER):
                                gtiles[kk] = g_t
                        g_t = gtiles[key]
                        half = key - gk
                        a_ap = g_t[:, 2 * half * F:(2 * half + 1) * F]
                        b_ap = g_t[:, (2 * half + 1) * F:(2 * half + 2) * F]
                    else:
                        a_t = pab.tile([128, F], f16, tag="A", name="at")
                        b_t = pab.tile([128, F], f16, tag="B", name="bt")
                        for side, dst in ((0, a_t), (1, b_t)):
                            nc.gpsimd.indirect_dma_start(
                                out=dst[:], out_offset=None, in_=xs_d[:],
                                in_offset=bass.IndirectOffsetOnAxis(
                                    ap=offs_t[:, 2 * key + side:
                                              2 * key + side + 1], axis=0))
                        a_ap, b_ap = a_t[:, :], b_t[:, :]
                elif l < 5:
                    a_ap = tiles[(l - 1, 2 * key)][:, :]
                    b_ap = tiles[(l - 1, 2 * key + 1)][:, :]
                elif l == 5:
                    a_ap = tiles[(4, 0)][:, :]
                    b_ap = tiles[(4, 1)][:, :]
                else:
                    a_ap = tiles['T5'][0:64, :]
                    b_ap = tiles['T5b'][:, :]

                # q = (a*alpha + beta) * b   [fused custom DVE op, 2x]
                # (offload experiments regressed: ops with upstream compute
                # deps stall the in-order Pool engine and block later gather
                # descgens -- keep Pool gather-only, all mixes on DVE)
                # (h-splitting the first mix / L5-L6 tail was tried and
                # regressed: extra per-op overhead + early-pipeline stalls
                # outweighed the ~1us tail-gap gain)
                hsp = ((0, 900), (900, 900)) if l >= 6 else ((0, F),)
                hsp_add = ((0, 900), (900, 900)) if l >= 5 else hsp
                p_t = ptmp.tile([n, F], f16, tag="p", name="p")
                _emit_custom(nc, "ANT_LC_MIX_PQ", p_t[:, :], a_ap, b_ap,
                             al, be)
                # output tile
                if l == 5:
                    r_t = pfin.tile([128, F], f16, tag="T5", name="t5")
                    tiles['T5'] = r_t
                elif l == 6:
                    r_t = pfin.tile([64, F], f16, tag="T6", name="t6")
                else:
                    pool = pt0 if l == 0 else plv
                    r_t = pool.tile([128, F], f16, tag=f"T{l}",
                                    name=f"t{l}_{key}")
                    tiles[(l, key)] = r_t

                if l == 5:
                    t5b = pfin.tile([64, F], f16, tag="T5b", name="t5b")
                    tiles['T5b'] = t5b
                for off, ln in hsp_add:
                    fs = slice(off, off + ln)
                    _emit_custom(nc, "ANT_LC_AFF_ADD", r_t[:, fs],
                                 a_ap[:, fs], p_t[:, fs], ga)
                    if l == 5:
                        # half-split so the T5b realign copy of half 0
                        # overlaps the add of half 1
                        nc.sync.dma_start(tiles['T5b'][:, fs],
                                          r_t[64:128, fs])
                    elif l == 6:
                        nc.sync.dma_start(out_d[:, fs], r_t[:, fs])
    nc.compile()
    _BASS_CACHE['nc'] = nc
    return nc


def _prep_inputs(x, idx_a, idx_b, ws):
    coef = _coef_tables(ws)
    offs = _offset_tables(idx_a, idx_b)
    x = np.ascontiguousarray(x, dtype=np.float32)
    in_maps = []
    for core in range(NCORES):
        xs = x[B2 * core:B2 * core + B2].transpose(1, 2, 3, 0)  # [C,H,W,B2]
        in_maps.append({"xs": _crop_table(xs).reshape(-1, 1),
                        "offs": offs, "coef": coef})
    return in_maps


def _assemble(core_outs, gamma):
    """core_outs: list of [64, F=(hh,ww,b)]; gamma [64] -> [16,64,900,1]."""
    full = np.stack([np.asarray(o, dtype=np.float32) for o in core_outs])
    full = full + gamma.astype(np.float32)[None, :, None]
    full = full.reshape(NCORES, K, P, B2)           # [core, k, p, b_local]
    full = full.transpose(0, 3, 1, 2).reshape(B, K, P, 1)
    return np.ascontiguousarray(full.astype(np.float32))


def kernel(x, idx_a, idx_b, w0, w1, w2, w3, w4, w5, w6):
    ws = [np.asarray(w, dtype=np.float32) for w in
          (w0, w1, w2, w3, w4, w5, w6)]
    x = np.asarray(x, dtype=np.float32)
    idx_a = np.asarray(idx_a, dtype=np.int32)
    idx_b = np.asarray(idx_b, dtype=np.int32)
    in_maps = _prep_inputs(x, idx_a, idx_b, ws)
    nc = _build_bass()
    from concourse.bass_utils import run_bass_kernel_spmd
    res = run_bass_kernel_spmd(nc, in_maps, core_ids=list(range(NCORES)))
    gamma = in_maps[0]["coef"][0:64, 4 * _NMIX]
    return _assemble([r["out"] for r in res.results], gamma)


def kernel_emulate(x, idx_a, idx_b, w0, w1, w2, w3, w4, w5, w6):
    """Pure-numpy emulation of the exact device schedule (debug aid)."""
    ws = [np.asarray(w, dtype=np.float32) for w in
          (w0, w1, w2, w3, w4, w5, w6)]
    in_maps = _prep_inputs(np.asarray(x, np.float32),
                           np.asarray(idx_a, np.int32),
                           np.asarray(idx_b, np.int32), ws)
    outs = [_emulate_core(m["xs"].reshape(NROW, ROWE), m["offs"], m["coef"])
            for m in in_maps]
    return _assemble(outs, in_maps[0]["coef"][0:64, 4 * _NMIX])
